# revision 44
# baseline (speedup 1.0000x reference)
"""Trainium2 Bass kernel for the EnergyBasedModel (equilibrium propagation)
negative-phase fixed-point iteration.

Strategy (pure data-parallel over batch, 8 cores), v2 "two-stream" design:
 - batch 8192 -> 1024 rows/core, FEATURE-MAJOR on chip ([feat, batch]),
   processed as TWO half-batch streams (n=0/1, 512 rows each) emitted
   phase-major ([L1 n0][L1 n1][L2 n0][L2 n1]): each stream's sigmoid
   dependency chains are hidden under the other stream's matmuls (engines
   execute in emission order, so stream-major would head-of-line block PE).
 - All large matmuls are fp8-e4m3 DoubleRow; weights pre-scaled by SC=256
   host-side, the 1/SC descale rides the custom-DVE op's immediate.
 - State/G buffers are PAIR-major 5D tiles ([128, pair, n, ko, 512]) so one
   custom-DVE op + one add + one sigmoid covers a whole DoubleRow pair per
   stream (halves DVE/ACT instruction counts vs per-k-tile ops) while
   slices stay contiguous for the interpreter and legal as DR rhs APs.
 - F1 = DT*SC*(sig(x)@W0 + b0) is loop-invariant, split at init into
   fp8 hi + fp8 lo residual and re-added into PSUM via a single DoubleRow
   matmul against an (I, I) stationary pair (ident_fp8).
 - per-step per-pair update: PSUM A = SC*DT*(ff+fb+b); custom-DVE computes
   bneg = (sig^2-sig)*A*(g^-(t+1)/SC); S += bneg on DVE (HW gpsimd ops
   carry ~1us hidden launch overhead each - keep Pool nearly empty);
   ScalarE recomputes sig -> fp8 per pair (early release of next-phase
   matmuls beats fewer/bigger ACT ops).
 - L3 is deferred into the next step's L1 phase; its -sc3*C3 cost term is a
   host-precomputed per-step C3S table streamed 20KB/step on the idle SP
   queue and applied as a plain Pool add (gpsimd cannot read PSUM and has
   no scalar_tensor_tensor on real HW - only the cost model allows them).
 - Measured on HW the PE is the wall at ~22us/step for the ~94 matmul
   instructions (about 2x the cost model; count-dominated - fp8-DR vs bf16
   idents time identically, and 1024-free matmuls are ISA-illegal).
"""

import os
import numpy as np
import ml_dtypes

BF16 = ml_dtypes.bfloat16
F8 = ml_dtypes.float8_e4m3fn

DT = 0.1
BETA = 0.1
N_STEPS = 20
NUM_CLASSES = 10

B_TOT = 8192
N_CORES = 8
B = B_TOT // N_CORES          # 1024 rows per core
NSB = 2                       # sub-batches
SB = B // NSB                 # 512: matmul free dim / psum bank
D1 = 1024
D2 = 512
D3 = NUM_CLASSES
K1 = D1 // 128                # 8 k-tiles of layer-1 features
K2 = D2 // 128                # 4 k-tiles of layer-2 features
JP1 = K1 // 2                 # 4 DoubleRow pairs over D1
JP2 = K2 // 2                 # 2 DoubleRow pairs over D2
SC = 256.0                    # fp8 weight pre-scale
SIG1 = 0.7310585786300049     # sigmoid(1.0)

_BUILT = None


def _register_sigprime_mul():
    """Fused custom-DVE op: out = (in0^2 - in0) * in1 * s0."""
    from concourse import dve_ops
    from concourse.dve_spec import Spec, Src0, Src1, C0, sq, lower
    from concourse.dve_spec import _has_src1
    from concourse.dve_uop import DveOpSpec

    name = "SIGPRIME_MUL_ANT"
    for op in dve_ops.OPS:
        if op.name == name:
            return op

    spec = Spec(
        body=(sq(Src0) - Src0) * Src1 * C0,
        reference=lambda in0, in1, s0, s1, imm2: (
            (in0.astype(np.float32) ** 2 - in0.astype(np.float32)) * in1 * s0),
    )
    row = dve_ops._CUSTOM_DVE_ROW_BASE + len(dve_ops.OPS)
    assert row < 0x20
    dve_ops._SUB_OPCODE_FOR_NAME[name] = row
    shas = {}
    for ver in ("v3", "v4"):
        shas[ver] = DveOpSpec(
            name=name, opcode=row, uops=lower(spec, ver=ver),
            rd1_en=_has_src1(spec)).sha(ver)
    op = dve_ops.DveOp(name, spec, subdim=False, uops_sha=shas,
                       perf_en={"v3": True, "v4": True})
    dve_ops.OPS.append(op)
    dve_ops.CUSTOM_DVE_SPECS[name] = spec
    return op


def _build(n_steps=N_STEPS, hw_reps=1, dve_adds_l1=(6, 7), dve_adds_l2=(2, 3),
           ps_bufs=3, scr_bufs=4, defer_l3=True, probe=None, dma_adds=False,
           split_l1=(), split_l2=(), crit_prio=0, crit_l1=(6, 7),
           crit_l2=(2, 3), l3_prio=0, split_sig=False, w2ta_first=False,
           l3_at=3, l1_order=None, shared_ps3=False, a_depth=2,
           b2r_first=False, c3_pool=False, l2_depth=2, s3_dve=False,
           l3_pool=False, c3_on="dve", l3_split=False):
    import concourse.bass as bass
    import concourse.mybir as mybir
    import concourse.tile as tile
    from concourse import bacc
    from concourse.masks import make_identity

    SIGP = _register_sigprime_mul()

    f32 = mybir.dt.float32
    bf16 = mybir.dt.bfloat16
    f16 = mybir.dt.float16
    f8 = mybir.dt.float8e4
    Alu = mybir.AluOpType
    Act = mybir.ActivationFunctionType
    DR = mybir.MatmulPerfMode.DoubleRow

    nc = bacc.Bacc("TRN2", target_bir_lowering=False, debug=False,
                   enable_asserts=False, num_devices=N_CORES)

    # ---- DRAM I/O ----
    xT_d = nc.dram_tensor("xT", [D1, B], bf16, kind="ExternalInput")
    w0_d = nc.dram_tensor("w0p", [JP1 * 128, 2, D1], f8, kind="ExternalInput")
    w1t_d = nc.dram_tensor("w1tp", [JP2 * 128, 2, D1], f8, kind="ExternalInput")
    w1_d = nc.dram_tensor("w1p", [JP1 * 128, 2, D2], f8, kind="ExternalInput")
    w2_d = nc.dram_tensor("w2p", [JP2 * 128, 2, 16], f8, kind="ExternalInput")
    w2ta_d = nc.dram_tensor("w2t_aug", [D3 + 1, D2], bf16, kind="ExternalInput")
    b2r_d = nc.dram_tensor("b2r", [1, D3], bf16, kind="ExternalInput")
    b0c_d = nc.dram_tensor("b0c", [D1, 1], f32, kind="ExternalInput")
    c3_d = nc.dram_tensor("c3", [D3, NSB, SB], f32, kind="ExternalInput")
    c3s_d = (nc.dram_tensor("c3s", [D3, N_STEPS, NSB, SB], f32,
                            kind="ExternalInput") if c3_pool else None)
    s1_d = nc.dram_tensor("s1", [D1, NSB, SB], f16, kind="ExternalOutput")
    s2_d = nc.dram_tensor("s2", [D2, NSB, SB], f16, kind="ExternalOutput")
    s3_d = nc.dram_tensor("s3", [D3, NSB, SB], f32, kind="ExternalOutput")

    with tile.TileContext(nc) as tc:
        with (
            tc.tile_pool(name="persist", bufs=1) as pp,
            tc.tile_pool(name="winit", bufs=1) as wip,
            tc.tile_pool(name="xin", bufs=4) as xp,
            tc.tile_pool(name="ps", bufs=ps_bufs, space="PSUM") as psp,
            tc.tile_pool(name="ps3", bufs=1, space="PSUM") as ps3p,
            tc.tile_pool(name="scr", bufs=scr_bufs) as scr,
        ):
            # ---- persistent weights ----
            W1T = [pp.tile([128, 2, D1], f8, tag=f"W1T_{j}", name=f"W1T_{j}")
                   for j in range(JP2)]
            W1 = [pp.tile([128, 2, D2], f8, tag=f"W1_{j}", name=f"W1_{j}")
                  for j in range(JP1)]
            # D3 padded to 16: DoubleRow LDWEIGHTS needs 16B-aligned ko stride
            W2 = [pp.tile([128, 2, 16], f8, tag=f"W2_{j}", name=f"W2_{j}")
                  for j in range(JP2)]
            W2TA = pp.tile([D3 + 1, D2], bf16, tag="W2TA", name="W2TA")
            B2R = pp.tile([1, D3], bf16, tag="B2R", name="B2R")
            ONES = pp.tile([1, NSB, SB], bf16, tag="ONES", name="ONES")
            IDENT = pp.tile([128, 128], bf16, tag="IDENT", name="IDENT")
            B0C = [pp.tile([128, 1], f32, tag=f"B0C_{m}", name=f"B0C_{m}")
                   for m in range(K1)]
            for j in range(JP2):
                nc.sync.dma_start(W1T[j][:], w1t_d[128 * j:128 * (j + 1)])
                nc.sync.dma_start(W2[j][:], w2_d[128 * j:128 * (j + 1)])
            for j in range(JP1):
                nc.sync.dma_start(W1[j][:], w1_d[128 * j:128 * (j + 1)])
            nc.sync.dma_start(W2TA[:], w2ta_d[:])
            nc.sync.dma_start(B2R[:], b2r_d[:])
            for m in range(K1):
                nc.sync.dma_start(B0C[m][:], b0c_d[128 * m:128 * (m + 1), :])
            nc.vector.memset(ONES[:], 1.0)
            make_identity(nc, IDENT[:])

            # ---- persistent state ----
            S1 = [pp.tile([128, NSB, SB], f16, tag=f"S1_{m}", name=f"S1_{m}")
                  for m in range(K1)]
            S2 = [pp.tile([128, NSB, SB], f16, tag=f"S2_{m}", name=f"S2_{m}")
                  for m in range(K2)]
            S3 = pp.tile([D3, NSB, SB], f32, tag="S3", name="S3")
            # G pair-buffers: [p, ko, n, c]; ko selects the k-subtile of a
            # DoubleRow pair, n the sub-batch.
            G1P = [pp.tile([128, 2, NSB, SB], f8, tag=f"G1P_{j}", name=f"G1P_{j}")
                   for j in range(JP1)]
            G2P = [pp.tile([128, 2, NSB, SB], f8, tag=f"G2P_{j}", name=f"G2P_{j}")
                   for j in range(JP2)]
            G3A = pp.tile([D3 + 1, NSB, SB], bf16, tag="G3A", name="G3A")
            F1 = [pp.tile([128, NSB, SB], bf16, tag=f"F1_{m}", name=f"F1_{m}")
                  for m in range(K1)]
            C3 = pp.tile([D3, NSB, SB], f32, tag="C3", name="C3")
            nc.sync.dma_start(C3[:], c3_d[:])
            # per-step pre-negated/pre-scaled C3: turns the layer-3 cost-term
            # update into a plain Pool tensor_add, freeing a DVE FIFO slot on
            # the critical path
            if c3_pool:
                C3S = pp.tile([D3, N_STEPS, NSB, SB], f32, tag="C3S",
                              name="C3S")
                nc.sync.dma_start(C3S[:], c3s_d[:])
            nc.vector.memset(S3[:], 1.0)
            nc.vector.memset(G3A[0:D3], SIG1)
            # ones row lives at partition 10: engines can't address a base
            # partition of 10, so fill it via SBUF->SBUF DMA
            nc.sync.dma_start(G3A[D3:D3 + 1], ONES[:])
            for m in range(K1):
                nc.vector.memset(S1[m][:], 1.0)
            for j in range(JP1):
                nc.vector.memset(G1P[j][:], SIG1)
            for m in range(K2):
                nc.vector.memset(S2[m][:], 1.0)
            for j in range(JP2):
                nc.vector.memset(G2P[j][:], SIG1)

            # ---- init: G0 = sig(xT) fp8 pairs, F1 = DT*SC*(W0^T G0 + b0) ----
            W0P = [wip.tile([128, 2, D1], f8, tag=f"W0P_{j}", name=f"W0P_{j}")
                   for j in range(JP1)]
            G0P = [wip.tile([128, 2, B], f8, tag=f"G0P_{j}", name=f"G0P_{j}")
                   for j in range(JP1)]
            for j in range(JP1):
                nc.sync.dma_start(W0P[j][:], w0_d[128 * j:128 * (j + 1)])
            for k in range(K1):
                xt = xp.tile([128, B], bf16, tag="xt", name="xt")
                nc.sync.dma_start(xt[:], xT_d[128 * k:128 * (k + 1), :])
                nc.scalar.activation(G0P[k // 2][:, k % 2, :], xt[:], Act.Sigmoid)
            for m in range(K1):
                ps = psp.tile([128, NSB, SB], f32, tag="ps", name="ps")
                for n in range(NSB):
                    for j in range(JP1):
                        nc.tensor.matmul(ps[:, n], W0P[j][:, :, 128 * m:128 * (m + 1)],
                                         G0P[j][:, :, SB * n:SB * (n + 1)],
                                         start=(j == 0), stop=(j == JP1 - 1),
                                         perf_mode=DR)
                nc.scalar.activation(F1[m][:], ps[:], Act.Identity,
                                     bias=B0C[m][:], scale=DT)

            # ---- main loop (scaled states) ----
            # Chip stores Shat_t = S_t / g^t (g=1.1 layers 1/2, 1.11 layer 3):
            # Shat += (-sig'*A) * g^-(t+1); sigmoid reads true S via ACT scale.
            g12 = 1.0 + DT
            g3 = 1.0 + DT + DT * BETA

            def l1_mm_a(t, m):
                ps = psp.tile([128, NSB, SB], f32, tag="ps", name="ps")
                for n in range(NSB):
                    nc.tensor.matmul(ps[:, n], IDENT[:], F1[m][:, n],
                                     start=True, stop=False)
                    nc.tensor.matmul(ps[:, n],
                                     W1T[0][:, :, 128 * m:128 * (m + 1)],
                                     G2P[0][:, :, n, :],
                                     start=False, stop=False, perf_mode=DR)
                return ps

            def l1_mm_b(t, m, ps):
                for n in range(NSB):
                    nc.tensor.matmul(ps[:, n],
                                     W1T[1][:, :, 128 * m:128 * (m + 1)],
                                     G2P[1][:, :, n, :],
                                     start=False, stop=True, perf_mode=DR)

            def _ew(t, m, ps, G, S, dve_adds, split):
                sc = g12 ** -(t + 1) / SC
                sg = g12 ** (t + 1)
                if probe == "mm_only":
                    drain = scr.tile([128, NSB, SB], bf16, tag="drain", name="drain")
                    nc.scalar.activation(drain[:], ps[:], Act.Identity)
                    return
                gsl = G[m // 2][:, m % 2]
                nslices = ([(slice(None), n) for n in range(NSB)]
                           if m in split else [(slice(None),)])
                for sl in nslices:
                    ix = (slice(None),) + sl[1:] if len(sl) > 1 else slice(None)
                    bneg = scr.tile([128, NSB, SB], f16, tag="bneg", name="bneg")
                    if len(sl) > 1:
                        n = sl[1]
                        bn = bneg[:, n]
                        nc.vector._custom_dve(SIGP, out=bn, in0=gsl[:, n],
                                              in1=ps[:, n], s0=sc)
                        if m in dve_adds:
                            nc.vector.tensor_add(S[m][:, n], S[m][:, n], bn)
                        elif dma_adds:
                            nc.gpsimd.dma_start(S[m][:, n], bn, accum_op=Alu.add)
                        else:
                            nc.gpsimd.tensor_add(S[m][:, n], S[m][:, n], bn)
                        nc.scalar.activation(gsl[:, n], S[m][:, n],
                                             Act.Sigmoid, scale=sg)
                    else:
                        nc.vector._custom_dve(SIGP, out=bneg[:], in0=gsl,
                                              in1=ps[:], s0=sc)
                        if m in dve_adds:
                            nc.vector.tensor_add(S[m][:], S[m][:], bneg[:])
                        elif dma_adds:
                            nc.gpsimd.dma_start(S[m][:], bneg[:], accum_op=Alu.add)
                        else:
                            nc.gpsimd.tensor_add(S[m][:], S[m][:], bneg[:])
                        if split_sig:
                            for n in range(NSB):
                                nc.scalar.activation(gsl[:, n], S[m][:, n],
                                                     Act.Sigmoid, scale=sg)
                        else:
                            nc.scalar.activation(gsl, S[m][:], Act.Sigmoid,
                                                 scale=sg)

            import contextlib

            def l1_ew(t, m, ps):
                cm = (tc.high_priority(offset=crit_prio)
                      if crit_prio and m in crit_l1 else contextlib.nullcontext())
                with cm:
                    _ew(t, m, ps, G1P, S1, dve_adds_l1, split_l1)

            def l2_head(t, m):
                ps = psp.tile([128, NSB, SB], f32, tag="ps", name="ps")
                for n in range(NSB):
                    if w2ta_first:
                        nc.tensor.matmul(ps[:, n],
                                         W2TA[:, 128 * m:128 * (m + 1)],
                                         G3A[:, n], start=True, stop=False)
                    for j in range(JP1 - 1):
                        nc.tensor.matmul(ps[:, n],
                                         W1[j][:, :, 128 * m:128 * (m + 1)],
                                         G1P[j][:, :, n, :],
                                         start=(j == 0 and not w2ta_first),
                                         stop=False, perf_mode=DR)
                return ps

            def l2_tail(t, m, ps):
                for n in range(NSB):
                    nc.tensor.matmul(ps[:, n],
                                     W1[JP1 - 1][:, :, 128 * m:128 * (m + 1)],
                                     G1P[JP1 - 1][:, :, n, :],
                                     start=False, stop=w2ta_first, perf_mode=DR)
                    if not w2ta_first:
                        nc.tensor.matmul(ps[:, n],
                                         W2TA[:, 128 * m:128 * (m + 1)],
                                         G3A[:, n], start=False, stop=True)

            def l2_ew(t, m, ps):
                cm = (tc.high_priority(offset=crit_prio)
                      if crit_prio and m in crit_l2 else contextlib.nullcontext())
                with cm:
                    _ew(t, m, ps, G2P, S2, dve_adds_l2, split_l2)

            def l3(t, ps3=None):
                cm = (tc.high_priority(offset=l3_prio) if l3_prio
                      else contextlib.nullcontext())
                with cm:
                    return _l3(t, ps3)

            def l3_open(t):
                pool3 = psp if shared_ps3 else ps3p
                ps3 = pool3.tile([D3, NSB, SB], f32,
                                 tag="ps" if shared_ps3 else "ps3", name="ps3")
                for n in range(NSB):
                    nc.tensor.matmul(ps3[:, n], B2R[:], ONES[:, n],
                                     start=True, stop=False)
                return ps3

            def _l3(t, ps3=None):
                sc3 = g3 ** -(t + 1)
                sg3 = g3 ** (t + 1)
                if ps3 is None:
                    pool3 = psp if shared_ps3 else ps3p
                    ps3 = pool3.tile([D3, NSB, SB], f32,
                                     tag="ps" if shared_ps3 else "ps3", name="ps3")
                for n in range(NSB):
                    nc.tensor.matmul(ps3[:, n], W2[0][:, :, 0:D3],
                                     G2P[0][:, :, n, :],
                                     start=(not b2r_first), stop=False,
                                     perf_mode=DR)
                    nc.tensor.matmul(ps3[:, n], W2[1][:, :, 0:D3],
                                     G2P[1][:, :, n, :],
                                     start=False, stop=b2r_first,
                                     perf_mode=DR)
                    if not b2r_first:
                        nc.tensor.matmul(ps3[:, n], B2R[:], ONES[:, n],
                                         start=False, stop=True)
                if probe == "mm_only":
                    drain3 = scr.tile([D3, NSB, SB], bf16, tag="drain3",
                                      name="drain3")
                    nc.scalar.activation(drain3[:], ps3[:], Act.Identity)
                    return
                bneg3 = scr.tile([D3, NSB, SB], f32, tag="bneg3", name="bneg3")
                if l3_pool:
                    # whole L3 elementwise chain on Pool, zero DVE ops:
                    #   t = (A*sc)*g;  bneg = (g-1)*t;  S3 += bneg;
                    #   S3 += -sc*C3
                    t3 = scr.tile([D3, NSB, SB], f32, tag="t3", name="t3")
                    nsl = list(range(NSB)) if l3_split else [slice(None)]
                    for n in nsl:
                        nc.gpsimd.scalar_tensor_tensor(
                            t3[:, n], ps3[:, n], sc3 / SC, G3A[0:D3, n],
                            op0=Alu.mult, op1=Alu.mult)
                        nc.gpsimd.scalar_tensor_tensor(
                            bneg3[:, n], G3A[0:D3, n], 1.0, t3[:, n],
                            op0=Alu.subtract, op1=Alu.mult)
                        nc.gpsimd.tensor_add(S3[:, n], S3[:, n], bneg3[:, n])
                        nc.gpsimd.scalar_tensor_tensor(
                            S3[:, n], C3[:, n], -sc3, S3[:, n],
                            op0=Alu.mult, op1=Alu.add)
                else:
                    nc.vector._custom_dve(SIGP, out=bneg3[:], in0=G3A[0:D3],
                                          in1=ps3[:], s0=sc3 / SC)
                    if dma_adds:
                        nc.gpsimd.dma_start(S3[:], bneg3[:], accum_op=Alu.add)
                    elif s3_dve:
                        nc.vector.tensor_add(S3[:], S3[:], bneg3[:])
                    else:
                        nc.gpsimd.tensor_add(S3[:], S3[:], bneg3[:])
                    if c3_pool:
                        nc.gpsimd.tensor_add(S3[:], S3[:],
                                             C3S[:, t % N_STEPS])
                    else:
                        eng3 = nc.gpsimd if c3_on == "pool" else nc.vector
                        eng3.scalar_tensor_tensor(S3[:], C3[:], -sc3,
                                                  S3[:], op0=Alu.mult,
                                                  op1=Alu.add)
                nc.scalar.activation(G3A[0:D3], S3[:], Act.Sigmoid, scale=sg3)

            loop_cm = (tc.For_i(0, hw_reps, 1) if hw_reps > 1
                       else contextlib.nullcontext())
            with loop_cm:
                l1o = list(l1_order) if l1_order else list(range(K1))
                ps3_pending = None
                for t in range(n_steps):
                    # staggered two-phase L1: phase A (ident+jp0) of tile
                    # m+2 is emitted before phase B (jp1) of tile m, giving
                    # ~3us of G2P[1]-independent PE work to hide the
                    # layer-2 m3 elementwise chain of the previous step.
                    ps_l1 = {l1o[i]: l1_mm_a(t, l1o[i]) for i in range(a_depth)}
                    for mi in range(K1):
                        m = l1o[mi]
                        if mi + a_depth < K1:
                            ps_l1[l1o[mi + a_depth]] = l1_mm_a(t, l1o[mi + a_depth])
                        l1_mm_b(t, m, ps_l1[m])
                        if defer_l3 and mi == l3_at and t > 0:
                            l3(t - 1, ps3_pending)
                            ps3_pending = None
                        l1_ew(t, m, ps_l1.pop(m))
                    ps_l2 = {i: l2_head(t, i) for i in range(l2_depth)}
                    for m in range(K2):
                        if m + l2_depth < K2:
                            ps_l2[m + l2_depth] = l2_head(t, m + l2_depth)
                        l2_tail(t, m, ps_l2[m])
                        l2_ew(t, m, ps_l2.pop(m))
                    if not defer_l3:
                        l3(t)
                    elif b2r_first:
                        # bank opener at the boundary: G-independent PE work
                        ps3_pending = l3_open(t)
                if defer_l3:
                    l3(n_steps - 1, ps3_pending)

            # ---- store raw scaled states; host rescales ----
            for m in range(K1):
                nc.sync.dma_start(s1_d[128 * m:128 * (m + 1)], S1[m][:])
            for m in range(K2):
                nc.sync.dma_start(s2_d[128 * m:128 * (m + 1)], S2[m][:])
            nc.sync.dma_start(s3_d[:], S3[:])

    nc.compile()
    return nc


def _build_v2(n_steps=N_STEPS, hw_reps=1, sig_big=True, l3_at=1,
              dve_pairs_l1=(), dve_pairs_l2=(), mm_depth=2, probe=None,
              l3_dve_c3=False, sig_l2_big=True, ident_fp8=False,
              pool_customs_l1=(), sig1_half=False, id_first=True,
              w2ta_head=True, b2r_head=True, l3_stage="dve",
              l3_all_dve=False, f1_act=()):
    """Two interleaved half-batch streams (n=0/1), pair-granularity state.

    Each weight-pair (DoubleRow pair) gets ONE custom-DVE op, ONE add and
    (optionally) a merged sigmoid per stream, halving instruction counts on
    DVE/ACT vs the per-tile v1. The n=1 stream's work fills the n=0
    stream's dependency-chain stalls and vice versa.
    """
    import concourse.bass as bass  # noqa
    import concourse.mybir as mybir
    import concourse.tile as tile
    from concourse import bacc
    from concourse.masks import make_identity

    SIGP = _register_sigprime_mul()

    f32 = mybir.dt.float32
    bf16 = mybir.dt.bfloat16
    f16 = mybir.dt.float16
    f8 = mybir.dt.float8e4
    Alu = mybir.AluOpType
    Act = mybir.ActivationFunctionType
    DR = mybir.MatmulPerfMode.DoubleRow

    nc = bacc.Bacc("TRN2", target_bir_lowering=False, debug=False,
                   enable_asserts=False, num_devices=N_CORES)

    xT_d = nc.dram_tensor("xT", [D1, B], bf16, kind="ExternalInput")
    w0_d = nc.dram_tensor("w0p", [JP1 * 128, 2, D1], f8, kind="ExternalInput")
    w1t_d = nc.dram_tensor("w1tp", [JP2 * 128, 2, D1], f8, kind="ExternalInput")
    w1_d = nc.dram_tensor("w1p", [JP1 * 128, 2, D2], f8, kind="ExternalInput")
    w2_d = nc.dram_tensor("w2p", [JP2 * 128, 2, 16], f8, kind="ExternalInput")
    w2ta_d = nc.dram_tensor("w2t_aug", [D3 + 1, D2], bf16, kind="ExternalInput")
    b2r_d = nc.dram_tensor("b2r", [1, D3], bf16, kind="ExternalInput")
    b0c_d = nc.dram_tensor("b0c", [D1, 1], f32, kind="ExternalInput")
    c3_d = nc.dram_tensor("c3", [D3, NSB, SB], f32, kind="ExternalInput")
    c3s_d = (None if l3_dve_c3 else
             nc.dram_tensor("c3s", [D3, N_STEPS, NSB, SB], bf16,
                            kind="ExternalInput"))
    s1_d = nc.dram_tensor("s1", [D1, NSB, SB], f16, kind="ExternalOutput")
    s2_d = nc.dram_tensor("s2", [D2, NSB, SB], f16, kind="ExternalOutput")
    s3_d = nc.dram_tensor("s3", [D3, NSB, SB], f32, kind="ExternalOutput")

    with tile.TileContext(nc) as tc:
        with (
            tc.tile_pool(name="persist", bufs=1) as pp,
            tc.tile_pool(name="winit", bufs=1) as wip,
            tc.tile_pool(name="xin", bufs=4) as xp,
            tc.tile_pool(name="ps", bufs=3, space="PSUM") as psp,
            tc.tile_pool(name="ps3", bufs=2, space="PSUM") as ps3p,
            tc.tile_pool(name="scr", bufs=4) as scr,
            tc.tile_pool(name="c3s", bufs=3) as c3sp,
        ):
            # ---- persistent weights ----
            W1T = [pp.tile([128, 2, D1], f8, tag=f"W1T_{j}", name=f"W1T_{j}")
                   for j in range(JP2)]
            W1 = [pp.tile([128, 2, D2], f8, tag=f"W1_{j}", name=f"W1_{j}")
                  for j in range(JP1)]
            W2 = [pp.tile([128, 2, 16], f8, tag=f"W2_{j}", name=f"W2_{j}")
                  for j in range(JP2)]
            W2TA = pp.tile([D3 + 1, D2], bf16, tag="W2TA", name="W2TA")
            B2R = pp.tile([1, D3], bf16, tag="B2R", name="B2R")
            ONES = pp.tile([1, NSB, SB], bf16, tag="ONES", name="ONES")
            IDENT = pp.tile([128, 128], bf16, tag="IDENT", name="IDENT")
            B0C = [pp.tile([128, 1], f32, tag=f"B0C_{m}", name=f"B0C_{m}")
                   for m in range(K1)]
            for j in range(JP2):
                nc.sync.dma_start(W1T[j][:], w1t_d[128 * j:128 * (j + 1)])
                nc.sync.dma_start(W2[j][:], w2_d[128 * j:128 * (j + 1)])
            for j in range(JP1):
                nc.sync.dma_start(W1[j][:], w1_d[128 * j:128 * (j + 1)])
            nc.sync.dma_start(W2TA[:], w2ta_d[:])
            nc.sync.dma_start(B2R[:], b2r_d[:])
            for m in range(K1):
                nc.sync.dma_start(B0C[m][:], b0c_d[128 * m:128 * (m + 1), :])
            nc.vector.memset(ONES[:], 1.0)
            make_identity(nc, IDENT[:])

            # ---- persistent state (pair-major layout) ----
            S1P = pp.tile([128, JP1, NSB, 2, SB], f16, tag="S1P", name="S1P")
            S2P = pp.tile([128, JP2, NSB, 2, SB], f16, tag="S2P", name="S2P")
            S3 = pp.tile([D3, NSB, SB], f32, tag="S3", name="S3")
            G1P = pp.tile([128, JP1, NSB, 2, SB], f8, tag="G1P", name="G1P")
            G2P = pp.tile([128, JP2, NSB, 2, SB], f8, tag="G2P", name="G2P")
            G3A = pp.tile([D3 + 1, NSB, SB], bf16, tag="G3A", name="G3A")
            if ident_fp8:
                F1HL = [pp.tile([128, 2, NSB, SB], f8, tag=f"F1HL_{m}",
                                name=f"F1HL_{m}") for m in range(K1)]
                IDENT2 = pp.tile([128, 2, 128], f8, tag="IDENT2",
                                 name="IDENT2")
            if not ident_fp8 or f1_act:
                F1 = [pp.tile([128, NSB, SB], bf16, tag=f"F1_{m}",
                              name=f"F1_{m}") for m in range(K1)]
            C3 = pp.tile([D3, NSB, SB], f32, tag="C3", name="C3")
            nc.sync.dma_start(C3[:], c3_d[:])
            nc.vector.memset(S3[:], 1.0)
            nc.vector.memset(G3A[0:D3], SIG1)
            nc.sync.dma_start(G3A[D3:D3 + 1], ONES[:])
            nc.vector.memset(S1P[:], 1.0)
            nc.vector.memset(G1P[:], SIG1)
            nc.vector.memset(S2P[:], 1.0)
            nc.vector.memset(G2P[:], SIG1)

            # ---- init: G0 = sig(xT) fp8 pairs, F1 = DT*SC*(W0^T G0 + b0) ----
            W0P = [wip.tile([128, 2, D1], f8, tag=f"W0P_{j}", name=f"W0P_{j}")
                   for j in range(JP1)]
            G0P = [wip.tile([128, 2, B], f8, tag=f"G0P_{j}", name=f"G0P_{j}")
                   for j in range(JP1)]
            for j in range(JP1):
                nc.sync.dma_start(W0P[j][:], w0_d[128 * j:128 * (j + 1)])
            for k in range(K1):
                xt = xp.tile([128, B], bf16, tag="xt", name="xt")
                nc.sync.dma_start(xt[:], xT_d[128 * k:128 * (k + 1), :])
                nc.scalar.activation(G0P[k // 2][:, k % 2, :], xt[:], Act.Sigmoid)
            if ident_fp8:
                nc.scalar.activation(IDENT2[:, 0], IDENT[:], Act.Identity)
                nc.scalar.activation(IDENT2[:, 1], IDENT[:], Act.Identity)
            for m in range(K1):
                ps = psp.tile([128, 2, SB], f32, tag="ps", name="ps")
                for n in range(NSB):
                    for j in range(JP1):
                        nc.tensor.matmul(ps[:, n], W0P[j][:, :, 128 * m:128 * (m + 1)],
                                         G0P[j][:, :, SB * n:SB * (n + 1)],
                                         start=(j == 0), stop=(j == JP1 - 1),
                                         perf_mode=DR)
                if ident_fp8:
                    f1f = scr.tile([128, NSB, SB], f32, tag="f1f", name="f1f")
                    nc.scalar.activation(f1f[:], ps[:], Act.Identity,
                                         bias=B0C[m][:], scale=DT)
                    nc.scalar.activation(F1HL[m][:, 0], ps[:], Act.Identity,
                                         bias=B0C[m][:], scale=DT)
                    nc.vector.tensor_sub(F1HL[m][:, 1], f1f[:],
                                         F1HL[m][:, 0])
                if not ident_fp8 or f1_act:
                    nc.scalar.activation(F1[m][:], ps[:], Act.Identity,
                                         bias=B0C[m][:], scale=DT)

            g12 = 1.0 + DT
            g3c = 1.0 + DT + DT * BETA
            # per-step C3S staging: one 20KB DMA per step on the idle SP
            # queue, issued a full step before l3(t) consumes it
            c3st = {}

            def stage_c3s(t):
                if l3_dve_c3 or t in c3st:
                    return
                tile_ = c3sp.tile([D3, NSB, SB], bf16, tag="c3st",
                                  name="c3st")
                nc.sync.dma_start(tile_[:], c3s_d[:, t])
                c3st[t] = tile_

            def l1_mm(t, jp, n):
                ps = psp.tile([128, 2, SB], f32, tag="ps", name="ps")
                # idents first: G-independent, keep PE busy/ramped while the
                # previous phase's sigmoids land
                def _ident(ko):
                    m = 2 * jp + ko
                    if jp in f1_act:
                        # ACT seeds the PSUM bank with F1; matmuls then
                        # accumulate (start=False) -- one less PE instr
                        nc.scalar.activation(ps[:, ko], F1[m][:, n],
                                             Act.Identity)
                    elif ident_fp8:
                        nc.tensor.matmul(ps[:, ko], IDENT2[:],
                                         F1HL[m][:, :, n, :],
                                         start=True, stop=False,
                                         perf_mode=DR)
                    else:
                        nc.tensor.matmul(ps[:, ko], IDENT[:], F1[m][:, n],
                                         start=True, stop=False)
                if id_first:
                    for ko in range(2):
                        _ident(ko)
                for ko in range(2):
                    m = 2 * jp + ko
                    if not id_first:
                        _ident(ko)
                    nc.tensor.matmul(ps[:, ko],
                                     W1T[0][:, :, 128 * m:128 * (m + 1)],
                                     G2P[:, 0, n],
                                     start=False, stop=False, perf_mode=DR)
                    nc.tensor.matmul(ps[:, ko],
                                     W1T[1][:, :, 128 * m:128 * (m + 1)],
                                     G2P[:, 1, n],
                                     start=False, stop=True, perf_mode=DR)
                return ps

            def l1_ew(t, jp, n, ps):
                sc = g12 ** -(t + 1) / SC
                sg = g12 ** (t + 1)
                if probe == "pe_only":
                    return
                if probe == "mm_only":
                    drain = scr.tile([128, 2, SB], bf16, tag="drain",
                                     name="drain")
                    nc.scalar.activation(drain[:], ps[:], Act.Identity)
                    return
                bneg = scr.tile([128, 2, SB], f16, tag="bneg", name="bneg")
                if jp in pool_customs_l1:
                    # decomposed on Pool: t = (A*sc)*g; bneg = (g-1)*t
                    tl = scr.tile([128, 2, SB], f16, tag="tl", name="tl")
                    nc.gpsimd.scalar_tensor_tensor(
                        tl[:], ps[:], sc, G1P[:, jp, n],
                        op0=Alu.mult, op1=Alu.mult)
                    nc.gpsimd.scalar_tensor_tensor(
                        bneg[:], G1P[:, jp, n], 1.0, tl[:],
                        op0=Alu.subtract, op1=Alu.mult)
                else:
                    nc.vector._custom_dve(SIGP, out=bneg[:],
                                          in0=G1P[:, jp, n],
                                          in1=ps[:], s0=sc)
                eng = nc.vector if jp in dve_pairs_l1 else nc.gpsimd
                eng.tensor_add(S1P[:, jp, n], S1P[:, jp, n],
                               bneg[:])
                if not sig_big and not sig1_half:
                    nc.scalar.activation(G1P[:, jp, n],
                                         S1P[:, jp, n],
                                         Act.Sigmoid, scale=sg)
                elif sig1_half and jp % 2 == 1:
                    nc.scalar.activation(G1P[:, jp - 1:jp + 1, n],
                                         S1P[:, jp - 1:jp + 1, n],
                                         Act.Sigmoid, scale=sg)

            def l2_mm(t, jq, n):
                ps = psp.tile([128, 2, SB], f32, tag="ps", name="ps")
                # W2TA first: needs only G3A (ready since l3 of the previous
                # step ran early in L1), so PE isn't blocked on sigma1
                if w2ta_head:
                    for ko in range(2):
                        m = 2 * jq + ko
                        nc.tensor.matmul(ps[:, ko],
                                         W2TA[:, 128 * m:128 * (m + 1)],
                                         G3A[:, n], start=True, stop=False)
                for ko in range(2):
                    m = 2 * jq + ko
                    for j in range(JP1):
                        nc.tensor.matmul(ps[:, ko],
                                         W1[j][:, :, 128 * m:128 * (m + 1)],
                                         G1P[:, j, n],
                                         start=(j == 0 and not w2ta_head),
                                         stop=(j == JP1 - 1 and w2ta_head),
                                         perf_mode=DR)
                    if not w2ta_head:
                        nc.tensor.matmul(ps[:, ko],
                                         W2TA[:, 128 * m:128 * (m + 1)],
                                         G3A[:, n], start=False, stop=True)
                return ps

            def l2_ew(t, jq, n, ps):
                sc = g12 ** -(t + 1) / SC
                sg = g12 ** (t + 1)
                if probe == "pe_only":
                    return
                if probe == "mm_only":
                    drain = scr.tile([128, 2, SB], bf16, tag="drain",
                                     name="drain")
                    nc.scalar.activation(drain[:], ps[:], Act.Identity)
                    return
                bneg = scr.tile([128, 2, SB], f16, tag="bneg", name="bneg")
                nc.vector._custom_dve(SIGP, out=bneg[:],
                                      in0=G2P[:, jq, n],
                                      in1=ps[:], s0=sc)
                eng = nc.vector if jq in dve_pairs_l2 else nc.gpsimd
                eng.tensor_add(S2P[:, jq, n], S2P[:, jq, n],
                               bneg[:])
                if not sig_l2_big:
                    nc.scalar.activation(G2P[:, jq, n],
                                         S2P[:, jq, n],
                                         Act.Sigmoid, scale=sg)

            def l3(t, n):
                sc3 = g3c ** -(t + 1)
                sg3 = g3c ** (t + 1)
                ps3 = ps3p.tile([D3, SB], f32, tag="ps3", name="ps3")
                if b2r_head:
                    nc.tensor.matmul(ps3[:], B2R[:], ONES[:, n],
                                     start=True, stop=False)
                nc.tensor.matmul(ps3[:], W2[0][:, :, 0:D3],
                                 G2P[:, 0, n],
                                 start=(not b2r_head), stop=False,
                                 perf_mode=DR)
                nc.tensor.matmul(ps3[:], W2[1][:, :, 0:D3],
                                 G2P[:, 1, n],
                                 start=False, stop=b2r_head, perf_mode=DR)
                if not b2r_head:
                    nc.tensor.matmul(ps3[:], B2R[:], ONES[:, n],
                                     start=False, stop=True)
                if probe == "pe_only":
                    return
                if probe == "mm_only":
                    drain3 = scr.tile([D3, SB], bf16, tag="drain3",
                                      name="drain3")
                    nc.scalar.activation(drain3[:], ps3[:], Act.Identity)
                    return
                bneg3 = scr.tile([D3, SB], f32, tag="bneg3", name="bneg3")
                # GPSIMD cannot access PSUM and has no scalar_tensor_tensor
                # on real HW: DVE custom op does the PSUM read, Pool does
                # plain adds only; the -sc3*C3 term comes from the host-
                # precomputed per-step C3S table.
                nc.vector._custom_dve(SIGP, out=bneg3[:],
                                      in0=G3A[0:D3, n],
                                      in1=ps3[:], s0=sc3 / SC)
                eng3 = nc.vector if l3_all_dve else nc.gpsimd
                eng3.tensor_add(S3[:, n], S3[:, n], bneg3[:])
                if l3_all_dve:
                    nc.vector.tensor_add(S3[:, n], S3[:, n], c3st[t][:, n])
                elif l3_dve_c3:
                    nc.vector.scalar_tensor_tensor(
                        S3[:, n], C3[:, n], -sc3, S3[:, n],
                        op0=Alu.mult, op1=Alu.add)
                else:
                    nc.gpsimd.tensor_add(S3[:, n], S3[:, n],
                                         c3st[t][:, n])
                nc.scalar.activation(G3A[0:D3, n], S3[:, n], Act.Sigmoid,
                                     scale=sg3)

            import contextlib
            loop_cm = (tc.For_i(0, hw_reps, 1) if hw_reps > 1
                       else contextlib.nullcontext())
            with loop_cm:
                for t in range(n_steps):
                    stage_c3s(t)
                    sg = g12 ** (t + 1)
                    # ---- L1, phase-major across the two streams: PE order
                    # is [L1n0 mms][L1n1 mms][L2n0 mms]... so stream n1's
                    # matmuls run while stream n0's sigmoids complete.
                    for n in range(NSB):
                        pend = {j: l1_mm(t, j, n) for j in range(mm_depth)}
                        for jp in range(JP1):
                            if jp + mm_depth < JP1:
                                pend[jp + mm_depth] = l1_mm(t, jp + mm_depth, n)
                            if jp == l3_at and t > 0:
                                l3(t - 1, n)
                            l1_ew(t, jp, n, pend.pop(jp))
                        if sig_big and probe != "mm_only":
                            nc.scalar.activation(G1P[:, :, n],
                                                 S1P[:, :, n],
                                                 Act.Sigmoid, scale=sg)
                    # ---- L2 phases ----
                    for n in range(NSB):
                        pend2 = {j: l2_mm(t, j, n) for j in range(JP2)}
                        for jq in range(JP2):
                            l2_ew(t, jq, n, pend2.pop(jq))
                        if sig_l2_big and probe != "mm_only":
                            nc.scalar.activation(G2P[:, :, n],
                                                 S2P[:, :, n],
                                                 Act.Sigmoid, scale=sg)
                for n in range(NSB):
                    l3(n_steps - 1, n)

            # ---- store raw scaled states; host rescales ----
            for m in range(K1):
                nc.sync.dma_start(s1_d[128 * m:128 * (m + 1)],
                                  S1P[:, m // 2, :, m % 2, :])
            for m in range(K2):
                nc.sync.dma_start(s2_d[128 * m:128 * (m + 1)],
                                  S2P[:, m // 2, :, m % 2, :])
            nc.sync.dma_start(s3_d[:], S3[:])

    nc.compile()
    return nc


# NOTE: dma_adds (SWDGE accum DMAs) measures fastest in the cost-model sim
# but wedges the axon-proxied runtime (mesh desync) -- do not enable.
BEST_CFG = dict(dve_adds_l1=(4, 5, 6, 7), dve_adds_l2=(1, 2, 3),
                split_l1=(6, 7), split_l2=(2, 3), l3_at=7)


BEST_V2 = dict(sig_big=False, sig_l2_big=False, w2ta_head=False,
               id_first=True, b2r_head=True,
               dve_pairs_l1=(0, 1, 2, 3), dve_pairs_l2=(0, 1),
               ident_fp8=True, mm_depth=3, l3_at=3)
USE_V2 = True


def build_best(n_steps=N_STEPS, hw_reps=1):
    if USE_V2:
        return _build_v2(n_steps, hw_reps=hw_reps, **BEST_V2)
    return _build(n_steps, hw_reps=hw_reps, **BEST_CFG)


def get_built(n_steps=N_STEPS):
    global _BUILT
    if _BUILT is None or _BUILT[0] != n_steps:
        _BUILT = (n_steps, build_best(n_steps))
    return _BUILT[1]


def _pair_pack(w, kdim, free):
    """[K*128, free] -> [npair*128, 2, free] DoubleRow stationary layout."""
    ktiles = w.reshape(kdim // 128, 128, free)
    npair = kdim // 256
    out = np.empty((npair * 128, 2, free), w.dtype)
    for j in range(npair):
        out[128 * j:128 * (j + 1), 0] = ktiles[2 * j]
        out[128 * j:128 * (j + 1), 1] = ktiles[2 * j + 1]
    return out


def _prep_core_inputs(x, target, W0, W1, W2, b0, b1, b2):
    """Host-side preprocessing -> list of per-core input dicts."""
    x = np.asarray(x, np.float32)
    target = np.asarray(target)
    W0 = np.asarray(W0, np.float32)
    W1 = np.asarray(W1, np.float32)
    W2 = np.asarray(W2, np.float32)
    b0 = np.asarray(b0, np.float32)
    b1 = np.asarray(b1, np.float32)
    b2 = np.asarray(b2, np.float32)

    w0p = _pair_pack((SC * W0).astype(F8), D1, D1)
    w1p = _pair_pack((SC * DT * W1).astype(F8), D1, D2)
    w1tp = _pair_pack(np.ascontiguousarray((SC * DT * W1).T).astype(F8), D2, D1)
    w2pad = np.zeros((D2, 16), np.float32)
    w2pad[:, :D3] = SC * DT * W2
    w2p = _pair_pack(w2pad.astype(F8), D2, 16)
    w2ta = np.concatenate([(SC * DT * W2).T, (SC * DT * b1)[None, :]],
                          axis=0).astype(BF16)
    b2r = (SC * DT * b2)[None, :].astype(BF16)
    b0c = (SC * DT * b0)[:, None].astype(np.float32)

    onehot = np.zeros((B_TOT, NUM_CLASSES), np.float32)
    onehot[np.arange(B_TOT), target.astype(np.int64)] = 1.0

    in_maps = []
    for c in range(N_CORES):
        sl = slice(c * B, (c + 1) * B)
        xT = np.ascontiguousarray(x[sl].T).astype(BF16)     # [1024, B]
        c3 = np.ascontiguousarray(
            (DT * BETA) * onehot[sl].T).reshape(D3, NSB, SB)
        g3 = 1.0 + DT + DT * BETA
        scales = np.array([-(g3 ** -(t + 1)) for t in range(N_STEPS)],
                          np.float32)
        c3s = np.ascontiguousarray(
            c3[:, None, :, :] * scales[None, :, None, None]).astype(BF16)
        in_maps.append({
            "xT": xT, "w0p": w0p, "w1p": w1p, "w1tp": w1tp, "w2p": w2p,
            "w2t_aug": w2ta, "b2r": b2r, "b0c": b0c, "c3": c3, "c3s": c3s,
        })
    return in_maps


_RUNNER = None


def _get_runner(nc):
    """Build the sharded PJRT callable once and reuse it across kernel()
    calls (run_bass_kernel_spmd re-jits + re-loads the NEFF every call)."""
    global _RUNNER
    if _RUNNER is not None:
        return _RUNNER
    import jax
    from jax.sharding import Mesh, PartitionSpec
    from jax.experimental.shard_map import shard_map
    import concourse.mybir as mybir
    from concourse.bass2jax import (_bass_exec_p, install_neuronx_cc_hook,
                                    partition_id_tensor)

    install_neuronx_cc_hook()
    partition_name = (nc.partition_id_tensor.name
                      if nc.partition_id_tensor else None)
    in_names, out_names, out_avals, zero_outs = [], [], [], []
    for alloc in nc.m.functions[0].allocations:
        if not isinstance(alloc, mybir.MemoryLocationSet):
            continue
        name = alloc.memorylocations[0].name
        if alloc.kind == "ExternalInput":
            if name != partition_name:
                in_names.append(name)
        elif alloc.kind == "ExternalOutput":
            shape = tuple(alloc.tensor_shape)
            dtype = mybir.dt.np(alloc.dtype)
            out_names.append(name)
            out_avals.append(jax.core.ShapedArray(shape, dtype))
            zero_outs.append(np.zeros(shape, dtype))
    n_params, n_outs = len(in_names), len(out_avals)
    all_names = in_names + out_names
    if partition_name is not None:
        all_names.append(partition_name)

    def _body(*args):
        operands = list(args)
        if partition_name is not None:
            operands.append(partition_id_tensor())
        return tuple(_bass_exec_p.bind(
            *operands, out_avals=tuple(out_avals), in_names=tuple(all_names),
            out_names=tuple(out_names), lowering_input_output_aliases=(),
            sim_require_finite=True, sim_require_nnan=True, nc=nc))

    devices = jax.devices()[:N_CORES]
    mesh = Mesh(np.asarray(devices), ("core",))
    in_specs = (PartitionSpec("core"),) * (n_params + n_outs)
    out_specs = (PartitionSpec("core"),) * n_outs
    fn = jax.jit(shard_map(_body, mesh=mesh, in_specs=in_specs,
                           out_specs=out_specs, check_rep=False),
                 donate_argnums=tuple(range(n_params, n_params + n_outs)),
                 keep_unused=True)

    def run(in_maps):
        per_core = [[np.asarray(m[name]) for name in in_names]
                    for m in in_maps]
        concat_in = [np.concatenate([per_core[c][i] for c in range(N_CORES)],
                                    axis=0) for i in range(n_params)]
        zeros = [np.zeros((N_CORES * z.shape[0], *z.shape[1:]), z.dtype)
                 for z in zero_outs]
        out = jax.block_until_ready(fn(*concat_in, *zeros))
        return [
            {name: np.asarray(out[i]).reshape(N_CORES, *out_avals[i].shape)[c]
             for i, name in enumerate(out_names)}
            for c in range(N_CORES)
        ]

    _RUNNER = run
    return run


def kernel(x, target, W0, W1, W2, b0, b1, b2):
    n_steps = int(os.environ.get("EBM_N_STEPS", N_STEPS))
    nc = get_built(n_steps)
    in_maps = _prep_core_inputs(x, target, W0, W1, W2, b0, b1, b2)
    try:
        results = _get_runner(nc)(in_maps)
    except Exception:
        from concourse import bass_utils
        results = bass_utils.run_bass_kernel_spmd(
            nc, in_maps, list(range(N_CORES))).results

    x = np.asarray(x, np.float32)
    g12n = (1.0 + DT) ** n_steps
    g3n = (1.0 + DT + DT * BETA) ** n_steps
    outs = []
    for c in range(N_CORES):
        r = results[c]
        sl = slice(c * B, (c + 1) * B)
        s1 = r["s1"].reshape(D1, B).astype(np.float32).T * g12n
        s2 = r["s2"].reshape(D2, B).astype(np.float32).T * g12n
        s3 = r["s3"].reshape(D3, B).astype(np.float32).T * g3n
        outs.append(np.concatenate([x[sl], s1, s2, s3], axis=1))
    return np.concatenate(outs, axis=0).astype(np.float32)



# revision 45
# speedup vs baseline: 1.1147x; 1.1147x over previous
"""Trainium2 Bass kernel for the EnergyBasedModel (equilibrium propagation)
negative-phase fixed-point iteration.

Strategy (pure data-parallel over batch, 8 cores), v2 "two-stream" design:
 - batch 8192 -> 1024 rows/core, FEATURE-MAJOR on chip ([feat, batch]),
   processed as TWO half-batch streams (n=0/1, 512 rows each) emitted
   phase-major ([L1 n0][L1 n1][L2 n0][L2 n1]): each stream's sigmoid
   dependency chains are hidden under the other stream's matmuls (engines
   execute in emission order, so stream-major would head-of-line block PE).
 - All large matmuls are fp8-e4m3 DoubleRow; weights pre-scaled by SC=256
   host-side, the 1/SC descale rides the custom-DVE op's immediate.
 - State/G buffers are PAIR-major 5D tiles ([128, pair, n, ko, 512]) so one
   custom-DVE op + one add + one sigmoid covers a whole DoubleRow pair per
   stream (halves DVE/ACT instruction counts vs per-k-tile ops) while
   slices stay contiguous for the interpreter and legal as DR rhs APs.
 - F1 = DT*SC*(sig(x)@W0 + b0) is loop-invariant, split at init into
   fp8 hi + fp8 lo residual and re-added into PSUM via a single DoubleRow
   matmul against an (I, I) stationary pair (ident_fp8).
 - per-step per-pair update: PSUM A = SC*DT*(ff+fb+b); custom-DVE computes
   bneg = (sig^2-sig)*A*(g^-(t+1)/SC); S += bneg on DVE (HW gpsimd ops
   carry ~1us hidden launch overhead each - keep Pool nearly empty);
   ScalarE recomputes sig -> fp8 per pair (early release of next-phase
   matmuls beats fewer/bigger ACT ops).
 - L3 is deferred into the next step's L1 phase; its -sc3*C3 cost term is a
   host-precomputed per-step C3S table streamed 20KB/step on the idle SP
   queue and applied as a plain Pool add (gpsimd cannot read PSUM and has
   no scalar_tensor_tensor on real HW - only the cost model allows them).
 - Measured on HW the PE is the wall at ~22us/step for the ~94 matmul
   instructions (about 2x the cost model; count-dominated - fp8-DR vs bf16
   idents time identically, and 1024-free matmuls are ISA-illegal).
"""

import os
import numpy as np
import ml_dtypes

BF16 = ml_dtypes.bfloat16
F8 = ml_dtypes.float8_e4m3fn

DT = 0.1
BETA = 0.1
N_STEPS = 20
NUM_CLASSES = 10

B_TOT = 8192
N_CORES = 8
B = B_TOT // N_CORES          # 1024 rows per core
NSB = 2                       # sub-batches
SB = B // NSB                 # 512: matmul free dim / psum bank
D1 = 1024
D2 = 512
D3 = NUM_CLASSES
K1 = D1 // 128                # 8 k-tiles of layer-1 features
K2 = D2 // 128                # 4 k-tiles of layer-2 features
JP1 = K1 // 2                 # 4 DoubleRow pairs over D1
JP2 = K2 // 2                 # 2 DoubleRow pairs over D2
SC = 256.0                    # fp8 weight pre-scale
SIG1 = 0.7310585786300049     # sigmoid(1.0)

_BUILT = None


def _register_sigprime_mul():
    """Fused custom-DVE op: out = (in0^2 - in0) * in1 * s0."""
    from concourse import dve_ops
    from concourse.dve_spec import Spec, Src0, Src1, C0, sq, lower
    from concourse.dve_spec import _has_src1
    from concourse.dve_uop import DveOpSpec

    name = "SIGPRIME_MUL_ANT"
    for op in dve_ops.OPS:
        if op.name == name:
            return op

    spec = Spec(
        body=(sq(Src0) - Src0) * Src1 * C0,
        reference=lambda in0, in1, s0, s1, imm2: (
            (in0.astype(np.float32) ** 2 - in0.astype(np.float32)) * in1 * s0),
    )
    row = dve_ops._CUSTOM_DVE_ROW_BASE + len(dve_ops.OPS)
    assert row < 0x20
    dve_ops._SUB_OPCODE_FOR_NAME[name] = row
    shas = {}
    for ver in ("v3", "v4"):
        shas[ver] = DveOpSpec(
            name=name, opcode=row, uops=lower(spec, ver=ver),
            rd1_en=_has_src1(spec)).sha(ver)
    op = dve_ops.DveOp(name, spec, subdim=False, uops_sha=shas,
                       perf_en={"v3": True, "v4": True})
    dve_ops.OPS.append(op)
    dve_ops.CUSTOM_DVE_SPECS[name] = spec
    return op


def _build(n_steps=N_STEPS, hw_reps=1, dve_adds_l1=(6, 7), dve_adds_l2=(2, 3),
           ps_bufs=3, scr_bufs=4, defer_l3=True, probe=None, dma_adds=False,
           split_l1=(), split_l2=(), crit_prio=0, crit_l1=(6, 7),
           crit_l2=(2, 3), l3_prio=0, split_sig=False, w2ta_first=False,
           l3_at=3, l1_order=None, shared_ps3=False, a_depth=2,
           b2r_first=False, c3_pool=False, l2_depth=2, s3_dve=False,
           l3_pool=False, c3_on="dve", l3_split=False):
    import concourse.bass as bass
    import concourse.mybir as mybir
    import concourse.tile as tile
    from concourse import bacc
    from concourse.masks import make_identity

    SIGP = _register_sigprime_mul()

    f32 = mybir.dt.float32
    bf16 = mybir.dt.bfloat16
    f16 = mybir.dt.float16
    f8 = mybir.dt.float8e4
    Alu = mybir.AluOpType
    Act = mybir.ActivationFunctionType
    DR = mybir.MatmulPerfMode.DoubleRow

    nc = bacc.Bacc("TRN2", target_bir_lowering=False, debug=False,
                   enable_asserts=False, num_devices=N_CORES)

    # ---- DRAM I/O ----
    xT_d = nc.dram_tensor("xT", [D1, B], bf16, kind="ExternalInput")
    w0_d = nc.dram_tensor("w0p", [JP1 * 128, 2, D1], f8, kind="ExternalInput")
    w1t_d = nc.dram_tensor("w1tp", [JP2 * 128, 2, D1], f8, kind="ExternalInput")
    w1_d = nc.dram_tensor("w1p", [JP1 * 128, 2, D2], f8, kind="ExternalInput")
    w2_d = nc.dram_tensor("w2p", [JP2 * 128, 2, 16], f8, kind="ExternalInput")
    w2ta_d = nc.dram_tensor("w2t_aug", [D3 + 1, D2], bf16, kind="ExternalInput")
    b2r_d = nc.dram_tensor("b2r", [1, D3], bf16, kind="ExternalInput")
    b0c_d = nc.dram_tensor("b0c", [D1, 1], f32, kind="ExternalInput")
    c3_d = nc.dram_tensor("c3", [D3, NSB, SB], f32, kind="ExternalInput")
    c3s_d = (nc.dram_tensor("c3s", [D3, N_STEPS, NSB, SB], f32,
                            kind="ExternalInput") if c3_pool else None)
    s1_d = nc.dram_tensor("s1", [D1, NSB, SB], f16, kind="ExternalOutput")
    s2_d = nc.dram_tensor("s2", [D2, NSB, SB], f16, kind="ExternalOutput")
    s3_d = nc.dram_tensor("s3", [D3, NSB, SB], f32, kind="ExternalOutput")

    with tile.TileContext(nc) as tc:
        with (
            tc.tile_pool(name="persist", bufs=1) as pp,
            tc.tile_pool(name="winit", bufs=1) as wip,
            tc.tile_pool(name="xin", bufs=4) as xp,
            tc.tile_pool(name="ps", bufs=ps_bufs, space="PSUM") as psp,
            tc.tile_pool(name="ps3", bufs=1, space="PSUM") as ps3p,
            tc.tile_pool(name="scr", bufs=scr_bufs) as scr,
        ):
            # ---- persistent weights ----
            W1T = [pp.tile([128, 2, D1], f8, tag=f"W1T_{j}", name=f"W1T_{j}")
                   for j in range(JP2)]
            W1 = [pp.tile([128, 2, D2], f8, tag=f"W1_{j}", name=f"W1_{j}")
                  for j in range(JP1)]
            # D3 padded to 16: DoubleRow LDWEIGHTS needs 16B-aligned ko stride
            W2 = [pp.tile([128, 2, 16], f8, tag=f"W2_{j}", name=f"W2_{j}")
                  for j in range(JP2)]
            W2TA = pp.tile([D3 + 1, D2], bf16, tag="W2TA", name="W2TA")
            B2R = pp.tile([1, D3], bf16, tag="B2R", name="B2R")
            ONES = pp.tile([1, NSB, SB], bf16, tag="ONES", name="ONES")
            IDENT = pp.tile([128, 128], bf16, tag="IDENT", name="IDENT")
            B0C = [pp.tile([128, 1], f32, tag=f"B0C_{m}", name=f"B0C_{m}")
                   for m in range(K1)]
            for j in range(JP2):
                nc.sync.dma_start(W1T[j][:], w1t_d[128 * j:128 * (j + 1)])
                nc.sync.dma_start(W2[j][:], w2_d[128 * j:128 * (j + 1)])
            for j in range(JP1):
                nc.sync.dma_start(W1[j][:], w1_d[128 * j:128 * (j + 1)])
            nc.sync.dma_start(W2TA[:], w2ta_d[:])
            nc.sync.dma_start(B2R[:], b2r_d[:])
            for m in range(K1):
                nc.sync.dma_start(B0C[m][:], b0c_d[128 * m:128 * (m + 1), :])
            nc.vector.memset(ONES[:], 1.0)
            make_identity(nc, IDENT[:])

            # ---- persistent state ----
            S1 = [pp.tile([128, NSB, SB], f16, tag=f"S1_{m}", name=f"S1_{m}")
                  for m in range(K1)]
            S2 = [pp.tile([128, NSB, SB], f16, tag=f"S2_{m}", name=f"S2_{m}")
                  for m in range(K2)]
            S3 = pp.tile([D3, NSB, SB], f32, tag="S3", name="S3")
            # G pair-buffers: [p, ko, n, c]; ko selects the k-subtile of a
            # DoubleRow pair, n the sub-batch.
            G1P = [pp.tile([128, 2, NSB, SB], f8, tag=f"G1P_{j}", name=f"G1P_{j}")
                   for j in range(JP1)]
            G2P = [pp.tile([128, 2, NSB, SB], f8, tag=f"G2P_{j}", name=f"G2P_{j}")
                   for j in range(JP2)]
            G3A = pp.tile([D3 + 1, NSB, SB], bf16, tag="G3A", name="G3A")
            F1 = [pp.tile([128, NSB, SB], bf16, tag=f"F1_{m}", name=f"F1_{m}")
                  for m in range(K1)]
            C3 = pp.tile([D3, NSB, SB], f32, tag="C3", name="C3")
            nc.sync.dma_start(C3[:], c3_d[:])
            # per-step pre-negated/pre-scaled C3: turns the layer-3 cost-term
            # update into a plain Pool tensor_add, freeing a DVE FIFO slot on
            # the critical path
            if c3_pool:
                C3S = pp.tile([D3, N_STEPS, NSB, SB], f32, tag="C3S",
                              name="C3S")
                nc.sync.dma_start(C3S[:], c3s_d[:])
            nc.vector.memset(S3[:], 1.0)
            nc.vector.memset(G3A[0:D3], SIG1)
            # ones row lives at partition 10: engines can't address a base
            # partition of 10, so fill it via SBUF->SBUF DMA
            nc.sync.dma_start(G3A[D3:D3 + 1], ONES[:])
            for m in range(K1):
                nc.vector.memset(S1[m][:], 1.0)
            for j in range(JP1):
                nc.vector.memset(G1P[j][:], SIG1)
            for m in range(K2):
                nc.vector.memset(S2[m][:], 1.0)
            for j in range(JP2):
                nc.vector.memset(G2P[j][:], SIG1)

            # ---- init: G0 = sig(xT) fp8 pairs, F1 = DT*SC*(W0^T G0 + b0) ----
            W0P = [wip.tile([128, 2, D1], f8, tag=f"W0P_{j}", name=f"W0P_{j}")
                   for j in range(JP1)]
            G0P = [wip.tile([128, 2, B], f8, tag=f"G0P_{j}", name=f"G0P_{j}")
                   for j in range(JP1)]
            for j in range(JP1):
                nc.sync.dma_start(W0P[j][:], w0_d[128 * j:128 * (j + 1)])
            for k in range(K1):
                xt = xp.tile([128, B], bf16, tag="xt", name="xt")
                nc.sync.dma_start(xt[:], xT_d[128 * k:128 * (k + 1), :])
                nc.scalar.activation(G0P[k // 2][:, k % 2, :], xt[:], Act.Sigmoid)
            for m in range(K1):
                ps = psp.tile([128, NSB, SB], f32, tag="ps", name="ps")
                for n in range(NSB):
                    for j in range(JP1):
                        nc.tensor.matmul(ps[:, n], W0P[j][:, :, 128 * m:128 * (m + 1)],
                                         G0P[j][:, :, SB * n:SB * (n + 1)],
                                         start=(j == 0), stop=(j == JP1 - 1),
                                         perf_mode=DR)
                nc.scalar.activation(F1[m][:], ps[:], Act.Identity,
                                     bias=B0C[m][:], scale=DT)

            # ---- main loop (scaled states) ----
            # Chip stores Shat_t = S_t / g^t (g=1.1 layers 1/2, 1.11 layer 3):
            # Shat += (-sig'*A) * g^-(t+1); sigmoid reads true S via ACT scale.
            g12 = 1.0 + DT
            g3 = 1.0 + DT + DT * BETA

            def l1_mm_a(t, m):
                ps = psp.tile([128, NSB, SB], f32, tag="ps", name="ps")
                for n in range(NSB):
                    nc.tensor.matmul(ps[:, n], IDENT[:], F1[m][:, n],
                                     start=True, stop=False)
                    nc.tensor.matmul(ps[:, n],
                                     W1T[0][:, :, 128 * m:128 * (m + 1)],
                                     G2P[0][:, :, n, :],
                                     start=False, stop=False, perf_mode=DR)
                return ps

            def l1_mm_b(t, m, ps):
                for n in range(NSB):
                    nc.tensor.matmul(ps[:, n],
                                     W1T[1][:, :, 128 * m:128 * (m + 1)],
                                     G2P[1][:, :, n, :],
                                     start=False, stop=True, perf_mode=DR)

            def _ew(t, m, ps, G, S, dve_adds, split):
                sc = g12 ** -(t + 1) / SC
                sg = g12 ** (t + 1)
                if probe == "mm_only":
                    drain = scr.tile([128, NSB, SB], bf16, tag="drain", name="drain")
                    nc.scalar.activation(drain[:], ps[:], Act.Identity)
                    return
                gsl = G[m // 2][:, m % 2]
                nslices = ([(slice(None), n) for n in range(NSB)]
                           if m in split else [(slice(None),)])
                for sl in nslices:
                    ix = (slice(None),) + sl[1:] if len(sl) > 1 else slice(None)
                    bneg = scr.tile([128, NSB, SB], f16, tag="bneg", name="bneg")
                    if len(sl) > 1:
                        n = sl[1]
                        bn = bneg[:, n]
                        nc.vector._custom_dve(SIGP, out=bn, in0=gsl[:, n],
                                              in1=ps[:, n], s0=sc)
                        if m in dve_adds:
                            nc.vector.tensor_add(S[m][:, n], S[m][:, n], bn)
                        elif dma_adds:
                            nc.gpsimd.dma_start(S[m][:, n], bn, accum_op=Alu.add)
                        else:
                            nc.gpsimd.tensor_add(S[m][:, n], S[m][:, n], bn)
                        nc.scalar.activation(gsl[:, n], S[m][:, n],
                                             Act.Sigmoid, scale=sg)
                    else:
                        nc.vector._custom_dve(SIGP, out=bneg[:], in0=gsl,
                                              in1=ps[:], s0=sc)
                        if m in dve_adds:
                            nc.vector.tensor_add(S[m][:], S[m][:], bneg[:])
                        elif dma_adds:
                            nc.gpsimd.dma_start(S[m][:], bneg[:], accum_op=Alu.add)
                        else:
                            nc.gpsimd.tensor_add(S[m][:], S[m][:], bneg[:])
                        if split_sig:
                            for n in range(NSB):
                                nc.scalar.activation(gsl[:, n], S[m][:, n],
                                                     Act.Sigmoid, scale=sg)
                        else:
                            nc.scalar.activation(gsl, S[m][:], Act.Sigmoid,
                                                 scale=sg)

            import contextlib

            def l1_ew(t, m, ps):
                cm = (tc.high_priority(offset=crit_prio)
                      if crit_prio and m in crit_l1 else contextlib.nullcontext())
                with cm:
                    _ew(t, m, ps, G1P, S1, dve_adds_l1, split_l1)

            def l2_head(t, m):
                ps = psp.tile([128, NSB, SB], f32, tag="ps", name="ps")
                for n in range(NSB):
                    if w2ta_first:
                        nc.tensor.matmul(ps[:, n],
                                         W2TA[:, 128 * m:128 * (m + 1)],
                                         G3A[:, n], start=True, stop=False)
                    for j in range(JP1 - 1):
                        nc.tensor.matmul(ps[:, n],
                                         W1[j][:, :, 128 * m:128 * (m + 1)],
                                         G1P[j][:, :, n, :],
                                         start=(j == 0 and not w2ta_first),
                                         stop=False, perf_mode=DR)
                return ps

            def l2_tail(t, m, ps):
                for n in range(NSB):
                    nc.tensor.matmul(ps[:, n],
                                     W1[JP1 - 1][:, :, 128 * m:128 * (m + 1)],
                                     G1P[JP1 - 1][:, :, n, :],
                                     start=False, stop=w2ta_first, perf_mode=DR)
                    if not w2ta_first:
                        nc.tensor.matmul(ps[:, n],
                                         W2TA[:, 128 * m:128 * (m + 1)],
                                         G3A[:, n], start=False, stop=True)

            def l2_ew(t, m, ps):
                cm = (tc.high_priority(offset=crit_prio)
                      if crit_prio and m in crit_l2 else contextlib.nullcontext())
                with cm:
                    _ew(t, m, ps, G2P, S2, dve_adds_l2, split_l2)

            def l3(t, ps3=None):
                cm = (tc.high_priority(offset=l3_prio) if l3_prio
                      else contextlib.nullcontext())
                with cm:
                    return _l3(t, ps3)

            def l3_open(t):
                pool3 = psp if shared_ps3 else ps3p
                ps3 = pool3.tile([D3, NSB, SB], f32,
                                 tag="ps" if shared_ps3 else "ps3", name="ps3")
                for n in range(NSB):
                    nc.tensor.matmul(ps3[:, n], B2R[:], ONES[:, n],
                                     start=True, stop=False)
                return ps3

            def _l3(t, ps3=None):
                sc3 = g3 ** -(t + 1)
                sg3 = g3 ** (t + 1)
                if ps3 is None:
                    pool3 = psp if shared_ps3 else ps3p
                    ps3 = pool3.tile([D3, NSB, SB], f32,
                                     tag="ps" if shared_ps3 else "ps3", name="ps3")
                for n in range(NSB):
                    nc.tensor.matmul(ps3[:, n], W2[0][:, :, 0:D3],
                                     G2P[0][:, :, n, :],
                                     start=(not b2r_first), stop=False,
                                     perf_mode=DR)
                    nc.tensor.matmul(ps3[:, n], W2[1][:, :, 0:D3],
                                     G2P[1][:, :, n, :],
                                     start=False, stop=b2r_first,
                                     perf_mode=DR)
                    if not b2r_first:
                        nc.tensor.matmul(ps3[:, n], B2R[:], ONES[:, n],
                                         start=False, stop=True)
                if probe == "mm_only":
                    drain3 = scr.tile([D3, NSB, SB], bf16, tag="drain3",
                                      name="drain3")
                    nc.scalar.activation(drain3[:], ps3[:], Act.Identity)
                    return
                bneg3 = scr.tile([D3, NSB, SB], f32, tag="bneg3", name="bneg3")
                if l3_pool:
                    # whole L3 elementwise chain on Pool, zero DVE ops:
                    #   t = (A*sc)*g;  bneg = (g-1)*t;  S3 += bneg;
                    #   S3 += -sc*C3
                    t3 = scr.tile([D3, NSB, SB], f32, tag="t3", name="t3")
                    nsl = list(range(NSB)) if l3_split else [slice(None)]
                    for n in nsl:
                        nc.gpsimd.scalar_tensor_tensor(
                            t3[:, n], ps3[:, n], sc3 / SC, G3A[0:D3, n],
                            op0=Alu.mult, op1=Alu.mult)
                        nc.gpsimd.scalar_tensor_tensor(
                            bneg3[:, n], G3A[0:D3, n], 1.0, t3[:, n],
                            op0=Alu.subtract, op1=Alu.mult)
                        nc.gpsimd.tensor_add(S3[:, n], S3[:, n], bneg3[:, n])
                        nc.gpsimd.scalar_tensor_tensor(
                            S3[:, n], C3[:, n], -sc3, S3[:, n],
                            op0=Alu.mult, op1=Alu.add)
                else:
                    nc.vector._custom_dve(SIGP, out=bneg3[:], in0=G3A[0:D3],
                                          in1=ps3[:], s0=sc3 / SC)
                    if dma_adds:
                        nc.gpsimd.dma_start(S3[:], bneg3[:], accum_op=Alu.add)
                    elif s3_dve:
                        nc.vector.tensor_add(S3[:], S3[:], bneg3[:])
                    else:
                        nc.gpsimd.tensor_add(S3[:], S3[:], bneg3[:])
                    if c3_pool:
                        nc.gpsimd.tensor_add(S3[:], S3[:],
                                             C3S[:, t % N_STEPS])
                    else:
                        eng3 = nc.gpsimd if c3_on == "pool" else nc.vector
                        eng3.scalar_tensor_tensor(S3[:], C3[:], -sc3,
                                                  S3[:], op0=Alu.mult,
                                                  op1=Alu.add)
                nc.scalar.activation(G3A[0:D3], S3[:], Act.Sigmoid, scale=sg3)

            loop_cm = (tc.For_i(0, hw_reps, 1) if hw_reps > 1
                       else contextlib.nullcontext())
            with loop_cm:
                l1o = list(l1_order) if l1_order else list(range(K1))
                ps3_pending = None
                for t in range(n_steps):
                    # staggered two-phase L1: phase A (ident+jp0) of tile
                    # m+2 is emitted before phase B (jp1) of tile m, giving
                    # ~3us of G2P[1]-independent PE work to hide the
                    # layer-2 m3 elementwise chain of the previous step.
                    ps_l1 = {l1o[i]: l1_mm_a(t, l1o[i]) for i in range(a_depth)}
                    for mi in range(K1):
                        m = l1o[mi]
                        if mi + a_depth < K1:
                            ps_l1[l1o[mi + a_depth]] = l1_mm_a(t, l1o[mi + a_depth])
                        l1_mm_b(t, m, ps_l1[m])
                        if defer_l3 and mi == l3_at and t > 0:
                            l3(t - 1, ps3_pending)
                            ps3_pending = None
                        l1_ew(t, m, ps_l1.pop(m))
                    ps_l2 = {i: l2_head(t, i) for i in range(l2_depth)}
                    for m in range(K2):
                        if m + l2_depth < K2:
                            ps_l2[m + l2_depth] = l2_head(t, m + l2_depth)
                        l2_tail(t, m, ps_l2[m])
                        l2_ew(t, m, ps_l2.pop(m))
                    if not defer_l3:
                        l3(t)
                    elif b2r_first:
                        # bank opener at the boundary: G-independent PE work
                        ps3_pending = l3_open(t)
                if defer_l3:
                    l3(n_steps - 1, ps3_pending)

            # ---- store raw scaled states; host rescales ----
            for m in range(K1):
                nc.sync.dma_start(s1_d[128 * m:128 * (m + 1)], S1[m][:])
            for m in range(K2):
                nc.sync.dma_start(s2_d[128 * m:128 * (m + 1)], S2[m][:])
            nc.sync.dma_start(s3_d[:], S3[:])

    nc.compile()
    return nc


def _build_v2(n_steps=N_STEPS, hw_reps=1, sig_big=True, l3_at=1,
              dve_pairs_l1=(), dve_pairs_l2=(), mm_depth=2, probe=None,
              l3_dve_c3=False, sig_l2_big=True, ident_fp8=False,
              pool_customs_l1=(), sig1_half=False, id_first=True,
              w2ta_head=True, b2r_head=True, l3_stage="dve",
              l3_all_dve=False, f1_act=(), scr_bufs=4):
    """Two interleaved half-batch streams (n=0/1), pair-granularity state.

    Each weight-pair (DoubleRow pair) gets ONE custom-DVE op, ONE add and
    (optionally) a merged sigmoid per stream, halving instruction counts on
    DVE/ACT vs the per-tile v1. The n=1 stream's work fills the n=0
    stream's dependency-chain stalls and vice versa.
    """
    import concourse.bass as bass  # noqa
    import concourse.mybir as mybir
    import concourse.tile as tile
    from concourse import bacc
    from concourse.masks import make_identity

    SIGP = _register_sigprime_mul()

    f32 = mybir.dt.float32
    bf16 = mybir.dt.bfloat16
    f16 = mybir.dt.float16
    f8 = mybir.dt.float8e4
    Alu = mybir.AluOpType
    Act = mybir.ActivationFunctionType
    DR = mybir.MatmulPerfMode.DoubleRow

    nc = bacc.Bacc("TRN2", target_bir_lowering=False, debug=False,
                   enable_asserts=False, num_devices=N_CORES)

    xT_d = nc.dram_tensor("xT", [D1, B], bf16, kind="ExternalInput")
    w0_d = nc.dram_tensor("w0p", [JP1 * 128, 2, D1], f8, kind="ExternalInput")
    w1t_d = nc.dram_tensor("w1tp", [JP2 * 128, 2, D1], f8, kind="ExternalInput")
    w1_d = nc.dram_tensor("w1p", [JP1 * 128, 2, D2], f8, kind="ExternalInput")
    w2_d = nc.dram_tensor("w2p", [JP2 * 128, 2, 16], f8, kind="ExternalInput")
    w2ta_d = nc.dram_tensor("w2t_aug", [D3 + 1, D2], bf16, kind="ExternalInput")
    b2r_d = nc.dram_tensor("b2r", [1, D3], bf16, kind="ExternalInput")
    b0c_d = nc.dram_tensor("b0c", [D1, 1], f32, kind="ExternalInput")
    c3_d = nc.dram_tensor("c3", [D3, NSB, SB], f32, kind="ExternalInput")
    c3s_d = (None if l3_dve_c3 else
             nc.dram_tensor("c3s", [D3, N_STEPS, NSB, SB], bf16,
                            kind="ExternalInput"))
    s1_d = nc.dram_tensor("s1", [D1, NSB, SB], f16, kind="ExternalOutput")
    s2_d = nc.dram_tensor("s2", [D2, NSB, SB], f16, kind="ExternalOutput")
    s3_d = nc.dram_tensor("s3", [D3, NSB, SB], f32, kind="ExternalOutput")

    with tile.TileContext(nc) as tc:
        with (
            tc.tile_pool(name="persist", bufs=1) as pp,
            tc.tile_pool(name="winit", bufs=1) as wip,
            tc.tile_pool(name="xin", bufs=4) as xp,
            tc.tile_pool(name="ps", bufs=3, space="PSUM") as psp,
            tc.tile_pool(name="ps3", bufs=2, space="PSUM") as ps3p,
            tc.tile_pool(name="scr", bufs=scr_bufs) as scr,
            tc.tile_pool(name="c3s", bufs=3) as c3sp,
        ):
            # ---- persistent weights ----
            W1T = [pp.tile([128, 2, D1], f8, tag=f"W1T_{j}", name=f"W1T_{j}")
                   for j in range(JP2)]
            W1 = [pp.tile([128, 2, D2], f8, tag=f"W1_{j}", name=f"W1_{j}")
                  for j in range(JP1)]
            W2 = [pp.tile([128, 2, 16], f8, tag=f"W2_{j}", name=f"W2_{j}")
                  for j in range(JP2)]
            W2TA = pp.tile([D3 + 1, D2], bf16, tag="W2TA", name="W2TA")
            B2R = pp.tile([1, D3], bf16, tag="B2R", name="B2R")
            ONES = pp.tile([1, NSB, SB], bf16, tag="ONES", name="ONES")
            IDENT = pp.tile([128, 128], bf16, tag="IDENT", name="IDENT")
            B0C = [pp.tile([128, 1], f32, tag=f"B0C_{m}", name=f"B0C_{m}")
                   for m in range(K1)]
            for j in range(JP2):
                nc.sync.dma_start(W1T[j][:], w1t_d[128 * j:128 * (j + 1)])
                nc.sync.dma_start(W2[j][:], w2_d[128 * j:128 * (j + 1)])
            for j in range(JP1):
                nc.sync.dma_start(W1[j][:], w1_d[128 * j:128 * (j + 1)])
            nc.sync.dma_start(W2TA[:], w2ta_d[:])
            nc.sync.dma_start(B2R[:], b2r_d[:])
            for m in range(K1):
                nc.sync.dma_start(B0C[m][:], b0c_d[128 * m:128 * (m + 1), :])
            nc.vector.memset(ONES[:], 1.0)
            make_identity(nc, IDENT[:])

            # ---- persistent state (pair-major layout) ----
            S1P = pp.tile([128, JP1, NSB, 2, SB], f16, tag="S1P", name="S1P")
            S2P = pp.tile([128, JP2, NSB, 2, SB], f16, tag="S2P", name="S2P")
            S3 = pp.tile([D3, NSB, SB], f32, tag="S3", name="S3")
            G1P = pp.tile([128, JP1, NSB, 2, SB], f8, tag="G1P", name="G1P")
            G2P = pp.tile([128, JP2, NSB, 2, SB], f8, tag="G2P", name="G2P")
            G3A = pp.tile([D3 + 1, NSB, SB], bf16, tag="G3A", name="G3A")
            if ident_fp8:
                F1HL = [pp.tile([128, 2, NSB, SB], f8, tag=f"F1HL_{m}",
                                name=f"F1HL_{m}") for m in range(K1)]
                IDENT2 = pp.tile([128, 2, 128], f8, tag="IDENT2",
                                 name="IDENT2")
            if not ident_fp8 or f1_act:
                F1 = [pp.tile([128, NSB, SB], bf16, tag=f"F1_{m}",
                              name=f"F1_{m}") for m in range(K1)]
            C3 = pp.tile([D3, NSB, SB], f32, tag="C3", name="C3")
            nc.sync.dma_start(C3[:], c3_d[:])
            nc.vector.memset(S3[:], 1.0)
            nc.vector.memset(G3A[0:D3], SIG1)
            nc.sync.dma_start(G3A[D3:D3 + 1], ONES[:])
            nc.vector.memset(S1P[:], 1.0)
            nc.vector.memset(G1P[:], SIG1)
            nc.vector.memset(S2P[:], 1.0)
            nc.vector.memset(G2P[:], SIG1)

            # ---- init: G0 = sig(xT) fp8 pairs, F1 = DT*SC*(W0^T G0 + b0) ----
            W0P = [wip.tile([128, 2, D1], f8, tag=f"W0P_{j}", name=f"W0P_{j}")
                   for j in range(JP1)]
            G0P = [wip.tile([128, 2, B], f8, tag=f"G0P_{j}", name=f"G0P_{j}")
                   for j in range(JP1)]
            for j in range(JP1):
                nc.sync.dma_start(W0P[j][:], w0_d[128 * j:128 * (j + 1)])
            for k in range(K1):
                xt = xp.tile([128, B], bf16, tag="xt", name="xt")
                nc.sync.dma_start(xt[:], xT_d[128 * k:128 * (k + 1), :])
                nc.scalar.activation(G0P[k // 2][:, k % 2, :], xt[:], Act.Sigmoid)
            if ident_fp8:
                nc.scalar.activation(IDENT2[:, 0], IDENT[:], Act.Identity)
                nc.scalar.activation(IDENT2[:, 1], IDENT[:], Act.Identity)
            for m in range(K1):
                ps = psp.tile([128, 2, SB], f32, tag="ps", name="ps")
                for n in range(NSB):
                    for j in range(JP1):
                        nc.tensor.matmul(ps[:, n], W0P[j][:, :, 128 * m:128 * (m + 1)],
                                         G0P[j][:, :, SB * n:SB * (n + 1)],
                                         start=(j == 0), stop=(j == JP1 - 1),
                                         perf_mode=DR)
                if ident_fp8:
                    f1f = scr.tile([128, NSB, SB], f32, tag="f1f", name="f1f")
                    nc.scalar.activation(f1f[:], ps[:], Act.Identity,
                                         bias=B0C[m][:], scale=DT)
                    nc.scalar.activation(F1HL[m][:, 0], ps[:], Act.Identity,
                                         bias=B0C[m][:], scale=DT)
                    nc.vector.tensor_sub(F1HL[m][:, 1], f1f[:],
                                         F1HL[m][:, 0])
                if not ident_fp8 or f1_act:
                    nc.scalar.activation(F1[m][:], ps[:], Act.Identity,
                                         bias=B0C[m][:], scale=DT)

            g12 = 1.0 + DT
            g3c = 1.0 + DT + DT * BETA
            # per-step C3S staging: one 20KB DMA per step on the idle SP
            # queue, issued a full step before l3(t) consumes it
            c3st = {}

            def stage_c3s(t):
                if l3_dve_c3 or t in c3st:
                    return
                tile_ = c3sp.tile([D3, NSB, SB], bf16, tag="c3st",
                                  name="c3st")
                nc.sync.dma_start(tile_[:], c3s_d[:, t])
                c3st[t] = tile_

            def l1_mm(t, jp, n):
                ps = psp.tile([128, 2, SB], f32, tag="ps", name="ps")
                # idents first: G-independent, keep PE busy/ramped while the
                # previous phase's sigmoids land
                def _ident(ko):
                    m = 2 * jp + ko
                    if jp in f1_act:
                        # ACT seeds the PSUM bank with F1; matmuls then
                        # accumulate (start=False) -- one less PE instr
                        nc.scalar.activation(ps[:, ko], F1[m][:, n],
                                             Act.Identity)
                    elif ident_fp8:
                        nc.tensor.matmul(ps[:, ko], IDENT2[:],
                                         F1HL[m][:, :, n, :],
                                         start=True, stop=False,
                                         perf_mode=DR)
                    else:
                        nc.tensor.matmul(ps[:, ko], IDENT[:], F1[m][:, n],
                                         start=True, stop=False)
                if id_first:
                    for ko in range(2):
                        _ident(ko)
                for ko in range(2):
                    m = 2 * jp + ko
                    if not id_first:
                        _ident(ko)
                    nc.tensor.matmul(ps[:, ko],
                                     W1T[0][:, :, 128 * m:128 * (m + 1)],
                                     G2P[:, 0, n],
                                     start=False, stop=False, perf_mode=DR)
                    nc.tensor.matmul(ps[:, ko],
                                     W1T[1][:, :, 128 * m:128 * (m + 1)],
                                     G2P[:, 1, n],
                                     start=False, stop=True, perf_mode=DR)
                return ps

            def l1_ew(t, jp, n, ps):
                sc = g12 ** -(t + 1) / SC
                sg = g12 ** (t + 1)
                if probe == "pe_only":
                    return
                if probe == "mm_only":
                    drain = scr.tile([128, 2, SB], bf16, tag="drain",
                                     name="drain")
                    nc.scalar.activation(drain[:], ps[:], Act.Identity)
                    return
                bneg = scr.tile([128, 2, SB], f16, tag="bneg", name="bneg")
                if jp in pool_customs_l1:
                    # decomposed on Pool: t = (A*sc)*g; bneg = (g-1)*t
                    tl = scr.tile([128, 2, SB], f16, tag="tl", name="tl")
                    nc.gpsimd.scalar_tensor_tensor(
                        tl[:], ps[:], sc, G1P[:, jp, n],
                        op0=Alu.mult, op1=Alu.mult)
                    nc.gpsimd.scalar_tensor_tensor(
                        bneg[:], G1P[:, jp, n], 1.0, tl[:],
                        op0=Alu.subtract, op1=Alu.mult)
                else:
                    nc.vector._custom_dve(SIGP, out=bneg[:],
                                          in0=G1P[:, jp, n],
                                          in1=ps[:], s0=sc)
                eng = nc.vector if jp in dve_pairs_l1 else nc.gpsimd
                eng.tensor_add(S1P[:, jp, n], S1P[:, jp, n],
                               bneg[:])
                if not sig_big and not sig1_half:
                    nc.scalar.activation(G1P[:, jp, n],
                                         S1P[:, jp, n],
                                         Act.Sigmoid, scale=sg)
                elif sig1_half and jp % 2 == 1:
                    nc.scalar.activation(G1P[:, jp - 1:jp + 1, n],
                                         S1P[:, jp - 1:jp + 1, n],
                                         Act.Sigmoid, scale=sg)

            def l2_mm(t, jq, n):
                ps = psp.tile([128, 2, SB], f32, tag="ps", name="ps")
                # W2TA first: needs only G3A (ready since l3 of the previous
                # step ran early in L1), so PE isn't blocked on sigma1
                if w2ta_head:
                    for ko in range(2):
                        m = 2 * jq + ko
                        nc.tensor.matmul(ps[:, ko],
                                         W2TA[:, 128 * m:128 * (m + 1)],
                                         G3A[:, n], start=True, stop=False)
                for ko in range(2):
                    m = 2 * jq + ko
                    for j in range(JP1):
                        nc.tensor.matmul(ps[:, ko],
                                         W1[j][:, :, 128 * m:128 * (m + 1)],
                                         G1P[:, j, n],
                                         start=(j == 0 and not w2ta_head),
                                         stop=(j == JP1 - 1 and w2ta_head),
                                         perf_mode=DR)
                    if not w2ta_head:
                        nc.tensor.matmul(ps[:, ko],
                                         W2TA[:, 128 * m:128 * (m + 1)],
                                         G3A[:, n], start=False, stop=True)
                return ps

            def l2_ew(t, jq, n, ps):
                sc = g12 ** -(t + 1) / SC
                sg = g12 ** (t + 1)
                if probe == "pe_only":
                    return
                if probe == "mm_only":
                    drain = scr.tile([128, 2, SB], bf16, tag="drain",
                                     name="drain")
                    nc.scalar.activation(drain[:], ps[:], Act.Identity)
                    return
                bneg = scr.tile([128, 2, SB], f16, tag="bneg", name="bneg")
                nc.vector._custom_dve(SIGP, out=bneg[:],
                                      in0=G2P[:, jq, n],
                                      in1=ps[:], s0=sc)
                eng = nc.vector if jq in dve_pairs_l2 else nc.gpsimd
                eng.tensor_add(S2P[:, jq, n], S2P[:, jq, n],
                               bneg[:])
                if not sig_l2_big:
                    nc.scalar.activation(G2P[:, jq, n],
                                         S2P[:, jq, n],
                                         Act.Sigmoid, scale=sg)

            def l3(t, n):
                sc3 = g3c ** -(t + 1)
                sg3 = g3c ** (t + 1)
                ps3 = ps3p.tile([D3, SB], f32, tag="ps3", name="ps3")
                if b2r_head:
                    nc.tensor.matmul(ps3[:], B2R[:], ONES[:, n],
                                     start=True, stop=False)
                nc.tensor.matmul(ps3[:], W2[0][:, :, 0:D3],
                                 G2P[:, 0, n],
                                 start=(not b2r_head), stop=False,
                                 perf_mode=DR)
                nc.tensor.matmul(ps3[:], W2[1][:, :, 0:D3],
                                 G2P[:, 1, n],
                                 start=False, stop=b2r_head, perf_mode=DR)
                if not b2r_head:
                    nc.tensor.matmul(ps3[:], B2R[:], ONES[:, n],
                                     start=False, stop=True)
                if probe == "pe_only":
                    return
                if probe == "mm_only":
                    drain3 = scr.tile([D3, SB], bf16, tag="drain3",
                                      name="drain3")
                    nc.scalar.activation(drain3[:], ps3[:], Act.Identity)
                    return
                bneg3 = scr.tile([D3, SB], f32, tag="bneg3", name="bneg3")
                # GPSIMD cannot access PSUM and has no scalar_tensor_tensor
                # on real HW: DVE custom op does the PSUM read, Pool does
                # plain adds only; the -sc3*C3 term comes from the host-
                # precomputed per-step C3S table.
                nc.vector._custom_dve(SIGP, out=bneg3[:],
                                      in0=G3A[0:D3, n],
                                      in1=ps3[:], s0=sc3 / SC)
                eng3 = nc.vector if l3_all_dve else nc.gpsimd
                eng3.tensor_add(S3[:, n], S3[:, n], bneg3[:])
                if l3_all_dve:
                    nc.vector.tensor_add(S3[:, n], S3[:, n], c3st[t][:, n])
                elif l3_dve_c3:
                    nc.vector.scalar_tensor_tensor(
                        S3[:, n], C3[:, n], -sc3, S3[:, n],
                        op0=Alu.mult, op1=Alu.add)
                else:
                    nc.gpsimd.tensor_add(S3[:, n], S3[:, n],
                                         c3st[t][:, n])
                nc.scalar.activation(G3A[0:D3, n], S3[:, n], Act.Sigmoid,
                                     scale=sg3)

            import contextlib
            loop_cm = (tc.For_i(0, hw_reps, 1) if hw_reps > 1
                       else contextlib.nullcontext())
            with loop_cm:
                for t in range(n_steps):
                    stage_c3s(t)
                    sg = g12 ** (t + 1)
                    # ---- L1, phase-major across the two streams: PE order
                    # is [L1n0 mms][L1n1 mms][L2n0 mms]... so stream n1's
                    # matmuls run while stream n0's sigmoids complete.
                    for n in range(NSB):
                        pend = {j: l1_mm(t, j, n) for j in range(mm_depth)}
                        for jp in range(JP1):
                            if jp + mm_depth < JP1:
                                pend[jp + mm_depth] = l1_mm(t, jp + mm_depth, n)
                            if jp == l3_at and t > 0:
                                l3(t - 1, n)
                            l1_ew(t, jp, n, pend.pop(jp))
                        if sig_big and probe != "mm_only":
                            nc.scalar.activation(G1P[:, :, n],
                                                 S1P[:, :, n],
                                                 Act.Sigmoid, scale=sg)
                    # ---- L2 phases ----
                    for n in range(NSB):
                        pend2 = {j: l2_mm(t, j, n) for j in range(JP2)}
                        for jq in range(JP2):
                            l2_ew(t, jq, n, pend2.pop(jq))
                        if sig_l2_big and probe != "mm_only":
                            nc.scalar.activation(G2P[:, :, n],
                                                 S2P[:, :, n],
                                                 Act.Sigmoid, scale=sg)
                for n in range(NSB):
                    l3(n_steps - 1, n)

            # ---- store raw scaled states; host rescales ----
            for m in range(K1):
                nc.sync.dma_start(s1_d[128 * m:128 * (m + 1)],
                                  S1P[:, m // 2, :, m % 2, :])
            for m in range(K2):
                nc.sync.dma_start(s2_d[128 * m:128 * (m + 1)],
                                  S2P[:, m // 2, :, m % 2, :])
            nc.sync.dma_start(s3_d[:], S3[:])

    nc.compile()
    return nc


# NOTE: dma_adds (SWDGE accum DMAs) measures fastest in the cost-model sim
# but wedges the axon-proxied runtime (mesh desync) -- do not enable.
BEST_CFG = dict(dve_adds_l1=(4, 5, 6, 7), dve_adds_l2=(1, 2, 3),
                split_l1=(6, 7), split_l2=(2, 3), l3_at=7)


BEST_V2 = dict(sig_big=False, sig_l2_big=False, w2ta_head=False,
               id_first=True, b2r_head=True,
               dve_pairs_l1=(0, 1, 2, 3), dve_pairs_l2=(0, 1),
               ident_fp8=True, mm_depth=3, l3_at=3)
USE_V2 = True


def build_best(n_steps=N_STEPS, hw_reps=1):
    if USE_V2:
        return _build_v2(n_steps, hw_reps=hw_reps, **BEST_V2)
    return _build(n_steps, hw_reps=hw_reps, **BEST_CFG)


def get_built(n_steps=N_STEPS):
    global _BUILT
    if _BUILT is None or _BUILT[0] != n_steps:
        _BUILT = (n_steps, build_best(n_steps))
    return _BUILT[1]


def _pair_pack(w, kdim, free):
    """[K*128, free] -> [npair*128, 2, free] DoubleRow stationary layout."""
    ktiles = w.reshape(kdim // 128, 128, free)
    npair = kdim // 256
    out = np.empty((npair * 128, 2, free), w.dtype)
    for j in range(npair):
        out[128 * j:128 * (j + 1), 0] = ktiles[2 * j]
        out[128 * j:128 * (j + 1), 1] = ktiles[2 * j + 1]
    return out


def _prep_core_inputs(x, target, W0, W1, W2, b0, b1, b2):
    """Host-side preprocessing -> list of per-core input dicts."""
    x = np.asarray(x, np.float32)
    target = np.asarray(target)
    W0 = np.asarray(W0, np.float32)
    W1 = np.asarray(W1, np.float32)
    W2 = np.asarray(W2, np.float32)
    b0 = np.asarray(b0, np.float32)
    b1 = np.asarray(b1, np.float32)
    b2 = np.asarray(b2, np.float32)

    w0p = _pair_pack((SC * W0).astype(F8), D1, D1)
    w1p = _pair_pack((SC * DT * W1).astype(F8), D1, D2)
    w1tp = _pair_pack(np.ascontiguousarray((SC * DT * W1).T).astype(F8), D2, D1)
    w2pad = np.zeros((D2, 16), np.float32)
    w2pad[:, :D3] = SC * DT * W2
    w2p = _pair_pack(w2pad.astype(F8), D2, 16)
    w2ta = np.concatenate([(SC * DT * W2).T, (SC * DT * b1)[None, :]],
                          axis=0).astype(BF16)
    b2r = (SC * DT * b2)[None, :].astype(BF16)
    b0c = (SC * DT * b0)[:, None].astype(np.float32)

    onehot = np.zeros((B_TOT, NUM_CLASSES), np.float32)
    onehot[np.arange(B_TOT), target.astype(np.int64)] = 1.0

    in_maps = []
    for c in range(N_CORES):
        sl = slice(c * B, (c + 1) * B)
        xT = np.ascontiguousarray(x[sl].T).astype(BF16)     # [1024, B]
        c3 = np.ascontiguousarray(
            (DT * BETA) * onehot[sl].T).reshape(D3, NSB, SB)
        g3 = 1.0 + DT + DT * BETA
        scales = np.array([-(g3 ** -(t + 1)) for t in range(N_STEPS)],
                          np.float32)
        c3s = np.ascontiguousarray(
            c3[:, None, :, :] * scales[None, :, None, None]).astype(BF16)
        in_maps.append({
            "xT": xT, "w0p": w0p, "w1p": w1p, "w1tp": w1tp, "w2p": w2p,
            "w2t_aug": w2ta, "b2r": b2r, "b0c": b0c, "c3": c3, "c3s": c3s,
        })
    return in_maps


_RUNNER = None


def _get_runner(nc):
    """Build the sharded PJRT callable once and reuse it across kernel()
    calls (run_bass_kernel_spmd re-jits + re-loads the NEFF every call)."""
    global _RUNNER
    if _RUNNER is not None:
        return _RUNNER
    import jax
    from jax.sharding import Mesh, PartitionSpec
    from jax.experimental.shard_map import shard_map
    import concourse.mybir as mybir
    from concourse.bass2jax import (_bass_exec_p, install_neuronx_cc_hook,
                                    partition_id_tensor)

    install_neuronx_cc_hook()
    partition_name = (nc.partition_id_tensor.name
                      if nc.partition_id_tensor else None)
    in_names, out_names, out_avals, zero_outs = [], [], [], []
    for alloc in nc.m.functions[0].allocations:
        if not isinstance(alloc, mybir.MemoryLocationSet):
            continue
        name = alloc.memorylocations[0].name
        if alloc.kind == "ExternalInput":
            if name != partition_name:
                in_names.append(name)
        elif alloc.kind == "ExternalOutput":
            shape = tuple(alloc.tensor_shape)
            dtype = mybir.dt.np(alloc.dtype)
            out_names.append(name)
            out_avals.append(jax.core.ShapedArray(shape, dtype))
            zero_outs.append(np.zeros(shape, dtype))
    n_params, n_outs = len(in_names), len(out_avals)
    all_names = in_names + out_names
    if partition_name is not None:
        all_names.append(partition_name)

    def _body(*args):
        operands = list(args)
        if partition_name is not None:
            operands.append(partition_id_tensor())
        return tuple(_bass_exec_p.bind(
            *operands, out_avals=tuple(out_avals), in_names=tuple(all_names),
            out_names=tuple(out_names), lowering_input_output_aliases=(),
            sim_require_finite=True, sim_require_nnan=True, nc=nc))

    devices = jax.devices()[:N_CORES]
    mesh = Mesh(np.asarray(devices), ("core",))
    in_specs = (PartitionSpec("core"),) * (n_params + n_outs)
    out_specs = (PartitionSpec("core"),) * n_outs
    fn = jax.jit(shard_map(_body, mesh=mesh, in_specs=in_specs,
                           out_specs=out_specs, check_rep=False),
                 donate_argnums=tuple(range(n_params, n_params + n_outs)),
                 keep_unused=True)

    def run(in_maps):
        per_core = [[np.asarray(m[name]) for name in in_names]
                    for m in in_maps]
        concat_in = [np.concatenate([per_core[c][i] for c in range(N_CORES)],
                                    axis=0) for i in range(n_params)]
        zeros = [np.zeros((N_CORES * z.shape[0], *z.shape[1:]), z.dtype)
                 for z in zero_outs]
        out = jax.block_until_ready(fn(*concat_in, *zeros))
        return [
            {name: np.asarray(out[i]).reshape(N_CORES, *out_avals[i].shape)[c]
             for i, name in enumerate(out_names)}
            for c in range(N_CORES)
        ]

    _RUNNER = run
    return run


def kernel(x, target, W0, W1, W2, b0, b1, b2):
    n_steps = int(os.environ.get("EBM_N_STEPS", N_STEPS))
    nc = get_built(n_steps)
    in_maps = _prep_core_inputs(x, target, W0, W1, W2, b0, b1, b2)
    try:
        results = _get_runner(nc)(in_maps)
    except Exception:
        from concourse import bass_utils
        results = bass_utils.run_bass_kernel_spmd(
            nc, in_maps, list(range(N_CORES))).results

    x = np.asarray(x, np.float32)
    g12n = (1.0 + DT) ** n_steps
    g3n = (1.0 + DT + DT * BETA) ** n_steps
    outs = []
    for c in range(N_CORES):
        r = results[c]
        sl = slice(c * B, (c + 1) * B)
        s1 = r["s1"].reshape(D1, B).astype(np.float32).T * g12n
        s2 = r["s2"].reshape(D2, B).astype(np.float32).T * g12n
        s3 = r["s3"].reshape(D3, B).astype(np.float32).T * g3n
        outs.append(np.concatenate([x[sl], s1, s2, s3], axis=1))
    return np.concatenate(outs, axis=0).astype(np.float32)

